# revision 4
# baseline (speedup 1.0000x reference)
"""Trainium2 Bass kernel for nn_DiagonalSSM (token-mix -> 2x [SAGE + diagonal SSM scan] -> proj).

Sharding: nodes (V) split across 8 cores; per-core dst-sorted edge chunks with
one-hot fp8 S tiles drive the segment-mean as PE matmuls on dma-gathered
source rows; inverse-degree applied afterwards on the mean via a broadcast
tensor_tensor multiply.

SSM state [128, 16 pairs, 2*VL] bf16 ordered k=(j2, m) so the per-step h add
is ONE broadcast tensor_tensor over the whole state (h appears once per pair).
The lam multiply is 32 per-tile tensor_scalars (lam varies per (k,p)). relu
emits fp8 tiles consumed by fp8 DoubleRow matmuls (2 k-tiles of contraction
per instruction, 0.5 cyc/row) with power-of-two scaling: wmix*128, rl*8,
descale 2^-10 folded into the PSUM->SBUF copy; w_res is scaled *1024 in bf16
so residual and mix share one PSUM accumulation group.
"""

import contextlib

import numpy as np
import ml_dtypes

import concourse.bacc as bacc
import concourse.bass as bass
import concourse.mybir as mybir
import concourse.tile as tile
from concourse.bass_utils import run_bass_kernel_spmd

BF16 = ml_dtypes.bfloat16
FP8 = ml_dtypes.float8_e4m3

NCORES = 8
RLSCALE = 8.0
WSCALE = 128.0
YDESCALE = 1.0 / (RLSCALE * WSCALE)


class Cfg:
    def __init__(self, T=8, V=10000, E=100000, CIN=128, H=256, DS=16, CO=64):
        self.T, self.V, self.E = T, V, E
        self.CIN, self.H, self.DS, self.CO = CIN, H, DS, CO
        self.VLOC = V // NCORES                      # real nodes per core
        self.VL = ((self.VLOC + 127) // 128) * 128   # padded local nodes
        self.NB = self.VL // 64                      # 64-node dst blocks
        self.ncb = None                              # chunks per block (set by prep)
        self.VCS = []                                # v-chunk windows (<=512)
        off = 0
        while off < self.VL:
            w = min(512, self.VL - off)
            self.VCS.append((off, w))
            off += w
        self.K = (H * DS) // 128                     # state tiles
        self.NP = self.K // 2                        # (j2) pairs
        self.MH = H // 128                           # output chunks of H
        self.CC = {0: max(1, CIN // 128), 1: H // 128}
        # engine split knobs
        self.R0_DVE = 5     # relus per L0 step on DVE (rest Act)
        self.TS1_ACT = 14   # L1 lam-mults per step on Act (rest DVE)


# ----------------------------------------------------------------------------
# host-side preparation
# ----------------------------------------------------------------------------

def prep_edges(cfg, edge_index):
    T = cfg.T
    VLOC, VL, NB = cfg.VLOC, cfg.VL, cfg.NB
    ei = np.asarray(edge_index)
    src_all, dst_all = ei[:, 0, :].astype(np.int64), ei[:, 1, :].astype(np.int64)

    buckets = [[[None] * NB for _ in range(T)] for _ in range(NCORES)]
    deg = np.zeros((T, NCORES, VL), np.float32)
    for t in range(T):
        s_t, d_t = src_all[t], dst_all[t]
        core = np.minimum(d_t // VLOC, NCORES - 1)
        for c in range(NCORES):
            m = core == c
            s_c, d_c = s_t[m], d_t[m] - c * VLOC
            np.add.at(deg[t, c], d_c, 1.0)
            b_c = d_c // 64
            order = np.argsort(b_c, kind="stable")
            s_c, d_c, b_c = s_c[order], d_c[order], b_c[order]
            bounds = np.searchsorted(b_c, np.arange(NB + 1))
            for b in range(NB):
                lo, hi = bounds[b], bounds[b + 1]
                buckets[c][t][b] = (s_c[lo:hi], d_c[lo:hi])

    ncb = 1
    for c in range(NCORES):
        for t in range(T):
            for b in range(NB):
                ncb = max(ncb, (len(buckets[c][t][b][0]) + 127) // 128)
    cfg.ncb = ncb
    nchunk = NB * ncb
    invdeg = 1.0 / np.maximum(deg, 1.0)

    src_rows = np.zeros((NCORES, T, nchunk, 128), np.int16)
    scol = np.full((NCORES, T, nchunk, 128), -1, np.int64)
    for c in range(NCORES):
        for t in range(T):
            for b in range(NB):
                s_b, d_b = buckets[c][t][b]
                n = len(s_b)
                pad = ncb * 128 - n
                rows = (s_b // VLOC) * VL + (s_b % VLOC)
                rows = np.concatenate([rows, np.zeros(pad, np.int64)])
                col = np.concatenate([d_b - b * 64, np.full(pad, -1, np.int64)])
                cs = b * ncb
                src_rows[c, t, cs:cs + ncb] = rows.reshape(ncb, 128).astype(np.int16)
                scol[c, t, cs:cs + ncb] = col.reshape(ncb, 128)
    return src_rows, scol, invdeg


def pack_gather_idx(cfg, src_rows):
    # [T, 128, G*64] int16; 1024 idxs per gather call = 16 partitions x 64
    # cols, replicated 8x across the 128 partitions.
    T = cfg.T
    nchunk = src_rows.shape[1]
    G = (nchunk + 7) // 8
    out = np.zeros((T, 128, G * 64), np.int16)
    for t in range(T):
        flat = np.zeros(G * 1024, np.int16)
        flat[:nchunk * 128] = src_rows[t].reshape(-1)
        out[t] = np.tile(flat.reshape(-1, 16).T, (8, 1))
    return out


# ----------------------------------------------------------------------------
# device program
# ----------------------------------------------------------------------------

def build_program(cfg, sim1=False):
    T, VL, CIN, H, DS, CO = cfg.T, cfg.VL, cfg.CIN, cfg.H, cfg.DS, cfg.CO
    K, NP, MH = cfg.K, cfg.NP, cfg.MH
    nchunk = cfg.NB * cfg.ncb
    G = (nchunk + 7) // 8
    fp32, bf16 = mybir.dt.float32, mybir.dt.bfloat16
    fp8, i16 = mybir.dt.float8e4, mybir.dt.int16
    AT = mybir.ActivationFunctionType
    OP = mybir.AluOpType
    DR = mybir.MatmulPerfMode.DoubleRow

    ndev = 1 if sim1 else NCORES
    nc = bacc.Bacc("TRN2", target_bir_lowering=False, debug=False,
                   num_devices=ndev)

    xs_in = nc.dram_tensor("xs_in", [T, CIN, VL], bf16, kind="ExternalInput")
    idx_in = nc.dram_tensor("idx_in", [T, 128, G * 64], i16, kind="ExternalInput")
    s8_in = nc.dram_tensor("s8_in", [T, 128, nchunk * 64], fp8, kind="ExternalInput")
    ivd_in = nc.dram_tensor("ivd_in", [T, 128, VL], bf16, kind="ExternalInput")
    wpre_diag_in = nc.dram_tensor("wpre_diag_in", [3, CIN, CIN], bf16, kind="ExternalInput")
    bpre_in = nc.dram_tensor("bpre_in", [CIN, 1], fp32, kind="ExternalInput")
    ident_in = nc.dram_tensor("ident_in", [128, 128], bf16, kind="ExternalInput")
    lam_in = nc.dram_tensor("lam_in", [2, 128, K], fp32, kind="ExternalInput")
    srl_in = nc.dram_tensor("srl_in", [2, 128, K], fp32, kind="ExternalInput")
    bsg_in = nc.dram_tensor("bsg_in", [2, 128, MH], fp32, kind="ExternalInput")
    wdr_in = nc.dram_tensor("wdr_in", [2, 128, NP, MH, 2, 128], fp8, kind="ExternalInput")
    wsage_in = nc.dram_tensor("wsage_in", [2, 2, 2, 128, H], bf16, kind="ExternalInput")
    wres_in = nc.dram_tensor("wres_in", [2, 2, 128, H], bf16, kind="ExternalInput")
    bmr_in = nc.dram_tensor("bmr_in", [2, 128, MH], fp32, kind="ExternalInput")
    wout_in = nc.dram_tensor("wout_in", [2, 128, CO], bf16, kind="ExternalInput")
    bout_in = nc.dram_tensor("bout_in", [64, 1], fp32, kind="ExternalInput")

    out_fm = nc.dram_tensor("out_fm", [CO, VL], fp32, kind="ExternalOutput")

    x1_T = nc.dram_tensor("x1_T", [T, 128, MH, VL], bf16)
    x0_nm = nc.dram_tensor("x0_nm", [T, VL, CIN], bf16)
    x1_nm = nc.dram_tensor("x1_nm", [T, VL, H], bf16)
    x0_full = nc.dram_tensor("x0_full", [T, NCORES * VL, CIN], bf16,
                             addr_space="Shared")
    x1_full = nc.dram_tensor("x1_full", [T, NCORES * VL, H], bf16,
                             addr_space="Shared")

    rg = [list(range(NCORES))]

    with tile.TileContext(nc) as tc, contextlib.ExitStack() as ctx:
        wpool = ctx.enter_context(tc.tile_pool(name="weights", bufs=1))
        lpool = ctx.enter_context(tc.tile_pool(name="layerw", bufs=1))
        spool = ctx.enter_context(tc.tile_pool(name="state", bufs=1))
        gpool = ctx.enter_context(tc.tile_pool(name="gather", bufs=2))
        spool8 = ctx.enter_context(tc.tile_pool(name="stiles", bufs=1))
        xpool = ctx.enter_context(tc.tile_pool(name="xt", bufs=2))
        hpool = ctx.enter_context(tc.tile_pool(name="hb", bufs=2))
        rpool = ctx.enter_context(tc.tile_pool(name="rl", bufs=1))
        mpool = ctx.enter_context(tc.tile_pool(name="misc", bufs=1))
        npool = ctx.enter_context(tc.tile_pool(name="nm", bufs=2))
        pp_y = ctx.enter_context(tc.tile_pool(name="py", bufs=1, space="PSUM"))
        pp_h = ctx.enter_context(tc.tile_pool(name="ph", bufs=1, space="PSUM"))
        pp_a = ctx.enter_context(tc.tile_pool(name="pa", bufs=1, space="PSUM"))

        # ---- persistent small weights
        wpre_d = wpool.tile([CIN, 3, CIN], bf16, tag="wpred")
        for tap in range(3):
            nc.sync.dma_start(out=wpre_d[:, tap, :], in_=wpre_diag_in[tap])
        bpre = wpool.tile([CIN, 1], fp32, tag="bpre")
        nc.sync.dma_start(out=bpre[:], in_=bpre_in[:])
        ident = wpool.tile([128, 128], bf16, tag="ident")
        nc.sync.dma_start(out=ident[:], in_=ident_in[:])
        lam_t = wpool.tile([128, 2, K], fp32, tag="lamt")
        srl_t = wpool.tile([128, 2, K], fp32, tag="srlt")
        bsg_t = wpool.tile([128, 2, MH], fp32, tag="bsgt")
        for L in range(2):
            nc.sync.dma_start(out=lam_t[:, L, :], in_=lam_in[L])
            nc.sync.dma_start(out=srl_t[:, L, :], in_=srl_in[L])
            nc.sync.dma_start(out=bsg_t[:, L, :], in_=bsg_in[L])
        wsage = wpool.tile([128, 2, 2, 2, H], bf16, tag="wsage")
        for L in range(2):
            for sn in range(2):
                for cc in range(2):
                    nc.sync.dma_start(out=wsage[:, L, sn, cc, :],
                                      in_=wsage_in[L, sn, cc])
        wres = wpool.tile([128, 2, 2, H], bf16, tag="wres")
        for L in range(2):
            for cc in range(2):
                nc.sync.dma_start(out=wres[:, L, cc, :], in_=wres_in[L, cc])
        bmr = wpool.tile([128, 2, MH], fp32, tag="bmr")
        for L in range(2):
            nc.sync.dma_start(out=bmr[:, L, :], in_=bmr_in[L])
        wout = wpool.tile([128, 2, CO], bf16, tag="wout")
        for cc in range(2):
            nc.sync.dma_start(out=wout[:, cc, :], in_=wout_in[cc])
        bout = wpool.tile([64, 1], fp32, tag="bout")
        nc.sync.dma_start(out=bout[:], in_=bout_in[:])

        # ---- x0 resident (feature-major, all t)
        x0sb = wpool.tile([128, T, VL], bf16, tag="x0sb")

        # ---- state; its flat bf16 view doubles as phase-A xs staging
        state = spool.tile([128, NP, 2 * VL], bf16, tag="state")
        sflat = state[:].rearrange("p a b -> p (a b)")

        # ---- phase A: token mix into x0sb, stage nm tables, AllGather per t
        for t in range(T):
            nc.sync.dma_start(out=sflat[:, t * VL:(t + 1) * VL], in_=xs_in[t])
        for t in range(T):
            sl = lambda u: sflat[:, u * VL:(u + 1) * VL]
            for (woff, wlen) in cfg.VCS:
                tm_ps = pp_y.tile([128, 512], fp32, tag="ya0", name="tmps")
                taps = [(tap, t + tap - 1) for tap in range(3)
                        if 0 <= t + tap - 1 < T]
                for i, (tap, u) in enumerate(taps):
                    nc.tensor.matmul(
                        out=tm_ps[:CIN, :wlen], lhsT=wpre_d[:, tap, :],
                        rhs=sl(u)[:, woff:woff + wlen],
                        start=(i == 0), stop=(i == len(taps) - 1))
                nc.scalar.activation(x0sb[:, t, woff:woff + wlen],
                                     tm_ps[:CIN, :wlen], AT.Identity,
                                     bias=bpre[:, 0:1], scale=1.0)
            nm = npool.tile([128, VL // 128, CIN], bf16, tag="nm")
            for bi in range(VL // 128):
                pt = pp_h.tile([128, 512], bf16, tag="h0", name="pt")
                nc.tensor.transpose(out=pt[:, :128],
                                    in_=x0sb[:, t, bi * 128:(bi + 1) * 128],
                                    identity=ident[:])
                nc.vector.tensor_copy(nm[:, bi, :CIN], pt[:, :CIN])
            nc.sync.dma_start(out=x0_nm[t].rearrange("(b p) c -> p b c", p=128),
                              in_=nm[:])
            if sim1:
                nc.sync.dma_start(out=x0_full[t, :VL, :], in_=x0_nm[t][:])
            else:
                nc.gpsimd.collective_compute(
                    "AllGather", OP.bypass, replica_groups=rg,
                    ins=[x0_nm[t][:]], outs=[x0_full[t][:]])

        def scan_layer(L):
            CC = cfg.CC[L]
            Cin = CIN if L == 0 else H
            xfull = x0_full if L == 0 else x1_full
            wdr = lpool.tile([128, NP, MH, 2, 128], fp8, tag="wdr")
            nc.sync.dma_start(out=wdr[:], in_=wdr_in[L])

            def prepare(t):
                # xt: feature-major input for res/self matmuls
                if L == 0:
                    xt = None  # use x0sb resident
                else:
                    xt = xpool.tile([128, MH, VL], bf16, tag="xin", name="xt")
                    nc.sync.dma_start(out=xt[:], in_=x1_T[t])
                xsl = (lambda cc: x0sb[:, t, :]) if L == 0 else \
                      (lambda cc: xt[:, cc, :])
                # gather + one-hot aggregation
                idx = gpool.tile([128, G * 64], i16, tag="idx", name="idx")
                nc.sync.dma_start(out=idx[:], in_=idx_in[t])
                s8 = spool8.tile([128, nchunk * 64], fp8, tag="soh", name="s8")
                nc.sync.dma_start(out=s8[:], in_=s8_in[t])
                ivd = gpool.tile([128, VL], bf16, tag="ivd", name="ivd")
                nc.sync.dma_start(out=ivd[:], in_=ivd_in[t])
                mean_sb = xpool.tile([128, 2, VL], bf16, tag="mean", name="mean_sb")
                win_of = {}
                for (woff, wlen) in cfg.VCS:
                    for b0 in range(woff // 64, (woff + wlen) // 64):
                        win_of[b0] = (woff, wlen)
                gt = None
                agg_ps = {}
                for ch in range(nchunk):
                    g, cg = divmod(ch, 8)
                    if cg == 0:
                        gt = gpool.tile([128, 8, Cin], bf16, tag="g", name="gt")
                        nc.gpsimd.dma_gather(
                            out_ap=gt[:], in_ap=xfull[t][:],
                            idxs_ap=idx[:, g * 64:(g + 1) * 64],
                            num_idxs=1024, num_idxs_reg=1024,
                            elem_size=Cin)
                    b, cb = divmod(ch, cfg.ncb)
                    woff, wlen = win_of[b]
                    if b % 8 == 0 and cb == 0:
                        agg_ps[0] = pp_a.tile([128, 512], fp32,
                                              tag="agg0", name="agg0")
                        if CC > 1:
                            # share the ya1 PSUM banks: L1 emits y only at
                            # t=7, after its prepare's aggs are done
                            agg_ps[1] = pp_y.tile([128, 512], fp32,
                                                  tag="ya1", name="agg1")
                    boff = b * 64 - woff
                    for cc in range(CC):
                        nc.tensor.matmul(
                            out=agg_ps[cc][:, boff:boff + 64],
                            lhsT=gt[:, cg, cc * 128:(cc + 1) * 128],
                            rhs=s8[:, ch * 64:(ch + 1) * 64], start=(cb == 0),
                            stop=(cb == cfg.ncb - 1))
                    if b == (woff + wlen) // 64 - 1 and cb == cfg.ncb - 1:
                        for cc in range(CC):
                            nc.scalar.activation(
                                mean_sb[:, cc, woff:woff + wlen],
                                agg_ps[cc][:, :wlen], AT.Copy)
                # mean *= invdeg (broadcast across cc)
                ms = mean_sb[:, :CC, :]
                nc.vector.tensor_tensor(
                    out=ms, in0=ms,
                    in1=ivd[:].unsqueeze(1).broadcast_to([128, CC, VL]),
                    op=OP.mult)
                # h = x@w_self + mean@w_neigh + bsg   -> h_sb [128, 2*VL]
                h_sb = hpool.tile([128, 2 * VL], bf16, tag="hsb", name="h_sb")
                for m in range(MH):
                    for (woff, wlen) in cfg.VCS:
                        h_ps = pp_h.tile([128, 512], fp32, tag="h0", name="h_ps")
                        for cc in range(CC):
                            nc.tensor.matmul(
                                out=h_ps[:, :wlen],
                                lhsT=wsage[:, L, 0, cc, m * 128:(m + 1) * 128],
                                rhs=xsl(cc)[:, woff:woff + wlen] if L == 0 else
                                    xt[:, cc, woff:woff + wlen],
                                start=(cc == 0), stop=False)
                        for cc in range(CC):
                            nc.tensor.matmul(
                                out=h_ps[:, :wlen],
                                lhsT=wsage[:, L, 1, cc, m * 128:(m + 1) * 128],
                                rhs=mean_sb[:, cc, woff:woff + wlen],
                                start=False, stop=(cc == CC - 1))
                        nc.scalar.activation(
                            h_sb[:, m * VL + woff:m * VL + woff + wlen],
                            h_ps[:, :wlen], AT.Identity,
                            bias=bsg_t[:, L, m:m + 1], scale=1.0)
                return xt, h_sb

            def state_k(k):
                j2, m = divmod(k, 2)
                return state[:, j2, m * VL:(m + 1) * VL]

            pre = prepare(0)
            h_prev = None
            for t in range(T):
                emit_y = (L == 0) or (t == T - 1)
                xt, h_sb = pre
                nxt = prepare(t + 1) if t + 1 < T else None

                # ---- state update (t=0 is implicit: state_0 = h_0)
                if t == 1:
                    for k in range(K):
                        nc.vector.tensor_scalar(
                            state_k(k), h_prev[:, (k % 2) * VL:(k % 2 + 1) * VL],
                            lam_t[:, L, k:k + 1], None, OP.mult)
                elif t > 1:
                    for k in range(K):
                        eng = nc.scalar if (L == 1 and k < cfg.TS1_ACT) else None
                        if eng is not None:
                            nc.scalar.activation(
                                state_k(k), state_k(k), AT.Copy,
                                scale=lam_t[:, L, k:k + 1])
                        else:
                            nc.vector.tensor_scalar(
                                state_k(k), state_k(k),
                                lam_t[:, L, k:k + 1], None, OP.mult)
                if t > 0:
                    st3 = state[:]
                    nc.vector.tensor_tensor(
                        out=st3, in0=st3,
                        in1=h_sb[:].unsqueeze(1).broadcast_to([128, NP, 2 * VL]),
                        op=OP.add)

                if emit_y:
                    # ---- relu -> fp8 rl pairs; y = res + DR mixes
                    yA = [pp_y.tile([128, 1024], fp32, tag=f"ya{m}", name=f"ya{m}")
                          for m in range(MH)]
                    yB = [pp_y.tile([128, 256], fp32, tag=f"yb{m}", name=f"yb{m}")
                          for m in range(MH)]
                    def ypsw(m, woff, wlen):
                        if woff + wlen <= 1024:
                            return yA[m][:, woff:woff + wlen]
                        return yB[m][:, :wlen]
                    for m in range(MH):
                        for (woff, wlen) in cfg.VCS:
                            for cc in range(CC):
                                nc.tensor.matmul(
                                    out=ypsw(m, woff, wlen),
                                    lhsT=wres[:, L, cc, m * 128:(m + 1) * 128],
                                    rhs=(x0sb[:, t, woff:woff + wlen] if L == 0
                                         else xt[:, cc, woff:woff + wlen]),
                                    start=(cc == 0), stop=False)
                    for j2 in range(NP):
                        rl = rpool.tile([128, 2, VL], fp8, tag=f"rl{j2 % 3}",
                                        name="rl")
                        for ko in range(2):
                            k = 2 * j2 + ko
                            src = (h_sb[:, ko * VL:(ko + 1) * VL] if t == 0
                                   else state_k(k))
                            if k < cfg.R0_DVE:
                                nc.vector.tensor_scalar(
                                    rl[:, ko, :], src, srl_t[:, L, k:k + 1],
                                    0.0, OP.mult, OP.max)
                            else:
                                nc.scalar.activation(
                                    rl[:, ko, :], src, AT.Relu,
                                    scale=srl_t[:, L, k:k + 1])
                        for m in range(MH):
                            for (woff, wlen) in cfg.VCS:
                                nc.tensor.matmul(
                                    out=ypsw(m, woff, wlen),
                                    lhsT=wdr[:, j2, m], perf_mode=DR,
                                    rhs=rl[:, :, woff:woff + wlen],
                                    start=False, stop=(j2 == NP - 1))
                    ys = mpool.tile([128, MH, VL], bf16, tag="ys")
                    for m in range(MH):
                        nc.scalar.activation(
                            ys[:, m, :1024], yA[m][:], AT.Identity,
                            bias=bmr[:, L, m:m + 1], scale=YDESCALE)
                        nc.scalar.activation(
                            ys[:, m, 1024:], yB[m][:, :VL - 1024], AT.Identity,
                            bias=bmr[:, L, m:m + 1], scale=YDESCALE)

                if L == 0:
                    nc.sync.dma_start(out=x1_T[t], in_=ys[:])
                    nm = npool.tile([128, VL // 128, H], bf16, tag="nm")
                    for bi in range(VL // 128):
                        for m in range(MH):
                            pt = pp_y.tile([128, 256], bf16, tag="yb0", name="pt")
                            nc.tensor.transpose(
                                out=pt[:, :128], in_=ys[:, m, bi * 128:(bi + 1) * 128],
                                identity=ident[:])
                            nc.vector.tensor_copy(
                                nm[:, bi, m * 128:(m + 1) * 128], pt[:, :128])
                    nc.sync.dma_start(
                        out=x1_nm[t].rearrange("(b p) c -> p b c", p=128),
                        in_=nm[:])
                    if sim1:
                        nc.sync.dma_start(out=x1_full[t, :VL, :], in_=x1_nm[t][:])
                    else:
                        nc.gpsimd.collective_compute(
                            "AllGather", OP.bypass, replica_groups=rg,
                            ins=[x1_nm[t][:]], outs=[x1_full[t][:]])
                if L == 1 and emit_y:
                    ot = mpool.tile([CO, VL], fp32, tag="outt")
                    for (woff, wlen) in cfg.VCS:
                        o_ps = pp_h.tile([CO, 512], fp32, tag="h0", name="o_ps")
                        for cc in range(MH):
                            nc.tensor.matmul(
                                out=o_ps[:, :wlen], lhsT=wout[:, cc, :],
                                rhs=ys[:, cc, woff:woff + wlen],
                                start=(cc == 0), stop=(cc == MH - 1))
                        nc.scalar.activation(ot[:, woff:woff + wlen],
                                             o_ps[:, :wlen], AT.Identity,
                                             bias=bout[:, 0:1], scale=1.0)
                    nc.sync.dma_start(out=out_fm[:], in_=ot[:])

                h_prev = h_sb
                pre = nxt
        scan_layer(0)
        scan_layer(1)

    nc.compile()
    return nc


# ----------------------------------------------------------------------------
# host wrapper
# ----------------------------------------------------------------------------

def make_inputs(cfg, inputs):
    T, CIN, H, DS, CO = cfg.T, cfg.CIN, cfg.H, cfg.DS, cfg.CO
    VLOC, VL, K, NP, MH = cfg.VLOC, cfg.VL, cfg.K, cfg.NP, cfg.MH
    xs = np.asarray(inputs["xs"], np.float32)
    src_rows, scol, invdeg = prep_edges(cfg, inputs["edge_index"])
    nchunk = cfg.NB * cfg.ncb

    w_pre = np.asarray(inputs["w_pre"], np.float32)
    wpre_diag = np.stack([np.diag(w_pre[:, tap]) for tap in range(3)]).astype(BF16)
    bpre = np.asarray(inputs["b_pre"], np.float32).reshape(CIN, 1)
    ident = np.eye(128, dtype=np.float32).astype(BF16)

    lam_a, srl_a, bsg_a, wdr_a, wsage_a, wres_a, bmr_a = [], [], [], [], [], [], []
    for L, f in ((0, CIN), (1, H)):
        lam_ij = np.exp(-np.exp(np.asarray(inputs[f"a_log{L}"], np.float64))) \
            .astype(np.float32)                                   # [H, DS]
        B_ij = np.asarray(inputs[f"B{L}"], np.float32)            # [H, DS]
        wm = np.asarray(inputs[f"w_mix{L}"], np.float32)          # [H*DS, H]
        # k = 2*j2 + m ; row p of tile k holds (i = m*128+p, j = j2)
        lam_k = np.zeros((128, K), np.float32)
        srl_k = np.zeros((128, K), np.float32)
        wdr_l = np.zeros((128, NP, MH, 2, 128), np.float32)
        p = np.arange(128)
        for j2 in range(NP):
            for m in range(2):
                i = m * 128 + p
                k = 2 * j2 + m
                lam_k[:, k] = lam_ij[i, j2]
                srl_k[:, k] = np.sign(B_ij[i, j2]) * RLSCALE
        for j2 in range(NP):
            for mh in range(MH):
                for ko in range(2):
                    i = ko * 128 + p
                    w_rows = wm[i * DS + j2, mh * 128:(mh + 1) * 128]  # [128,128]
                    w_rows = w_rows * np.abs(B_ij[i, j2])[:, None] * WSCALE
                    wdr_l[:, j2, mh, ko, :] = w_rows
        lam_a.append(lam_k)
        srl_a.append(srl_k)
        wdr_a.append(wdr_l.astype(FP8))
        bsg_a.append(np.asarray(inputs[f"b_sage{L}"], np.float32)
                     .reshape(MH, 128).T)
        ws = np.zeros((2, 2, 128, H), np.float32)
        wsf = np.asarray(inputs[f"w_self{L}"], np.float32)
        wnf = np.asarray(inputs[f"w_neigh{L}"], np.float32)
        for cc in range((f + 127) // 128):
            ws[0, cc] = wsf[cc * 128:(cc + 1) * 128]
            ws[1, cc] = wnf[cc * 128:(cc + 1) * 128]
        wsage_a.append(ws.astype(BF16))
        wr = np.zeros((2, 128, H), np.float32)
        wrf = np.asarray(inputs[f"w_res{L}"], np.float32) * (RLSCALE * WSCALE)
        for cc in range((f + 127) // 128):
            wr[cc] = wrf[cc * 128:(cc + 1) * 128]
        wres_a.append(wr.astype(BF16))
        bmr_a.append((np.asarray(inputs[f"b_res{L}"], np.float32)
                      + np.asarray(inputs[f"b_mix{L}"], np.float32))
                     .reshape(MH, 128).T)

    wout = np.asarray(inputs["w_out"], np.float32).reshape(MH, 128, CO).astype(BF16)
    bout = np.asarray(inputs["b_out"], np.float32).reshape(CO, 1)
    wcol = np.arange(64)

    in_maps = []
    for c in range(NCORES):
        onehot = (scol[c][..., None] == wcol)                 # [T, nchunk, 128, 64]
        s8 = np.ascontiguousarray(
            onehot.transpose(0, 2, 1, 3).reshape(T, 128, nchunk * 64)
        ).astype(FP8)
        sh = xs[:, c * VLOC:(c + 1) * VLOC, :]
        xs_sh = np.zeros((T, CIN, VL), np.float32)
        xs_sh[:, :, :VLOC] = np.transpose(sh, (0, 2, 1))
        ivd = np.broadcast_to(invdeg[:, c][:, None, :], (T, 128, VL))
        in_maps.append({
            "xs_in": xs_sh.astype(BF16),
            "idx_in": pack_gather_idx(cfg, src_rows[c]),
            "s8_in": s8,
            "ivd_in": np.ascontiguousarray(ivd).astype(BF16),
            "wpre_diag_in": wpre_diag,
            "bpre_in": bpre,
            "ident_in": ident,
            "lam_in": np.stack(lam_a),
            "srl_in": np.stack(srl_a),
            "bsg_in": np.stack(bsg_a).astype(np.float32),
            "wdr_in": np.stack(wdr_a),
            "wsage_in": np.stack(wsage_a),
            "wres_in": np.stack(wres_a),
            "bmr_in": np.stack(bmr_a).astype(np.float32),
            "wout_in": wout,
            "bout_in": bout,
        })
    return in_maps


_CACHED = {}


def kernel(**inputs):
    cfg = Cfg()
    in_maps = make_inputs(cfg, inputs)
    key = ("full", cfg.ncb)
    if key not in _CACHED:
        _CACHED[key] = build_program(cfg)
    nc = _CACHED[key]
    res = run_bass_kernel_spmd(nc, in_maps, list(range(NCORES)))
    out = np.zeros((cfg.V, cfg.CO), np.float32)
    for c in range(NCORES):
        out[c * cfg.VLOC:(c + 1) * cfg.VLOC] = \
            res.results[c]["out_fm"][:, :cfg.VLOC].T
    return out


# revision 6
# speedup vs baseline: 1.1784x; 1.1784x over previous
"""Trainium2 Bass kernel for nn_DiagonalSSM (token-mix -> 2x [SAGE + diagonal SSM scan] -> proj).

Sharding: nodes (V) split across 8 cores; per-core dst-sorted edge chunks with
one-hot fp8 S tiles drive the segment-mean as PE matmuls on dma-gathered
source rows; inverse-degree applied afterwards on the mean via a broadcast
tensor_tensor multiply.

SSM state [128, 16 pairs, 2*VL] bf16 ordered k=(j2, m) so the per-step h add
is ONE broadcast tensor_tensor over the whole state (h appears once per pair).
The lam multiply is 32 per-tile tensor_scalars (lam varies per (k,p)). relu
emits fp8 tiles consumed by fp8 DoubleRow matmuls (2 k-tiles of contraction
per instruction, 0.5 cyc/row) with power-of-two scaling: wmix*128, rl*8,
descale 2^-10 folded into the PSUM->SBUF copy; w_res is scaled *1024 in bf16
so residual and mix share one PSUM accumulation group.
"""

import contextlib

import numpy as np
import ml_dtypes

import concourse.bacc as bacc
import concourse.bass as bass
import concourse.mybir as mybir
import concourse.tile as tile
from concourse.bass_utils import run_bass_kernel_spmd

BF16 = ml_dtypes.bfloat16
FP8 = ml_dtypes.float8_e4m3

NCORES = 8
RLSCALE = 8.0
WSCALE = 128.0
YDESCALE = 1.0 / (RLSCALE * WSCALE)


class Cfg:
    def __init__(self, T=8, V=10000, E=100000, CIN=128, H=256, DS=16, CO=64):
        self.T, self.V, self.E = T, V, E
        self.CIN, self.H, self.DS, self.CO = CIN, H, DS, CO
        self.VLOC = V // NCORES                      # real nodes per core
        self.VL = ((self.VLOC + 127) // 128) * 128   # padded local nodes
        self.NB = self.VL // 64                      # 64-node dst blocks
        self.ncb = None                              # chunks per block (set by prep)
        self.VCS = []                                # v-chunk windows (<=512)
        off = 0
        while off < self.VL:
            w = min(512, self.VL - off)
            self.VCS.append((off, w))
            off += w
        self.K = (H * DS) // 128                     # state tiles
        self.NP = self.K // 2                        # (j2) pairs
        self.MH = H // 128                           # output chunks of H
        self.CC = {0: max(1, CIN // 128), 1: H // 128}
        # engine split knobs
        self.R_DVE = 1      # relus per 8 k-tiles on DVE
        self.R_POOL = 2     # relus per 8 k-tiles on GPSIMD (rest on Act)
        self.TS1_ACT = 0    # L1 lam-mults per step on Act (rest DVE)
        self.NGRP = 4       # pair-groups per state update (pipelining)


# ----------------------------------------------------------------------------
# host-side preparation
# ----------------------------------------------------------------------------

def prep_edges(cfg, edge_index):
    T = cfg.T
    VLOC, VL, NB = cfg.VLOC, cfg.VL, cfg.NB
    ei = np.asarray(edge_index)
    src_all, dst_all = ei[:, 0, :].astype(np.int64), ei[:, 1, :].astype(np.int64)

    buckets = [[[None] * NB for _ in range(T)] for _ in range(NCORES)]
    deg = np.zeros((T, NCORES, VL), np.float32)
    for t in range(T):
        s_t, d_t = src_all[t], dst_all[t]
        core = np.minimum(d_t // VLOC, NCORES - 1)
        for c in range(NCORES):
            m = core == c
            s_c, d_c = s_t[m], d_t[m] - c * VLOC
            np.add.at(deg[t, c], d_c, 1.0)
            b_c = d_c // 64
            order = np.argsort(b_c, kind="stable")
            s_c, d_c, b_c = s_c[order], d_c[order], b_c[order]
            bounds = np.searchsorted(b_c, np.arange(NB + 1))
            for b in range(NB):
                lo, hi = bounds[b], bounds[b + 1]
                buckets[c][t][b] = (s_c[lo:hi], d_c[lo:hi])

    ncb = 1
    for c in range(NCORES):
        for t in range(T):
            for b in range(NB):
                ncb = max(ncb, (len(buckets[c][t][b][0]) + 127) // 128)
    cfg.ncb = ncb
    nchunk = NB * ncb
    invdeg = 1.0 / np.maximum(deg, 1.0)

    src_rows = np.zeros((NCORES, T, nchunk, 128), np.int16)
    scol = np.full((NCORES, T, nchunk, 128), -1, np.int64)
    for c in range(NCORES):
        for t in range(T):
            for b in range(NB):
                s_b, d_b = buckets[c][t][b]
                n = len(s_b)
                pad = ncb * 128 - n
                rows = (s_b // VLOC) * VL + (s_b % VLOC)
                rows = np.concatenate([rows, np.zeros(pad, np.int64)])
                col = np.concatenate([d_b - b * 64, np.full(pad, -1, np.int64)])
                cs = b * ncb
                src_rows[c, t, cs:cs + ncb] = rows.reshape(ncb, 128).astype(np.int16)
                scol[c, t, cs:cs + ncb] = col.reshape(ncb, 128)
    return src_rows, scol, invdeg


def pack_gather_idx(cfg, src_rows):
    # [T, 128, G*64] int16; 1024 idxs per gather call = 16 partitions x 64
    # cols, replicated 8x across the 128 partitions.
    T = cfg.T
    nchunk = src_rows.shape[1]
    G = (nchunk + 7) // 8
    out = np.zeros((T, 128, G * 64), np.int16)
    for t in range(T):
        flat = np.zeros(G * 1024, np.int16)
        flat[:nchunk * 128] = src_rows[t].reshape(-1)
        out[t] = np.tile(flat.reshape(-1, 16).T, (8, 1))
    return out


# ----------------------------------------------------------------------------
# device program
# ----------------------------------------------------------------------------

def build_program(cfg, sim1=False):
    T, VL, CIN, H, DS, CO = cfg.T, cfg.VL, cfg.CIN, cfg.H, cfg.DS, cfg.CO
    K, NP, MH = cfg.K, cfg.NP, cfg.MH
    nchunk = cfg.NB * cfg.ncb
    G = (nchunk + 7) // 8
    fp32, bf16 = mybir.dt.float32, mybir.dt.bfloat16
    fp8, i16 = mybir.dt.float8e4, mybir.dt.int16
    AT = mybir.ActivationFunctionType
    OP = mybir.AluOpType
    DR = mybir.MatmulPerfMode.DoubleRow

    ndev = 1 if sim1 else NCORES
    nc = bacc.Bacc("TRN2", target_bir_lowering=False, debug=False,
                   num_devices=ndev)

    xs_in = nc.dram_tensor("xs_in", [T, CIN, VL], bf16, kind="ExternalInput")
    idx_in = nc.dram_tensor("idx_in", [T, 128, G * 64], i16, kind="ExternalInput")
    s8_in = nc.dram_tensor("s8_in", [T, 128, nchunk * 64], fp8, kind="ExternalInput")
    ivd_in = nc.dram_tensor("ivd_in", [T, 128, VL], bf16, kind="ExternalInput")
    wpre_diag_in = nc.dram_tensor("wpre_diag_in", [3, CIN, CIN], bf16, kind="ExternalInput")
    bpre_in = nc.dram_tensor("bpre_in", [CIN, 1], fp32, kind="ExternalInput")
    ident_in = nc.dram_tensor("ident_in", [128, 128], bf16, kind="ExternalInput")
    lam_in = nc.dram_tensor("lam_in", [2, 128, K], fp32, kind="ExternalInput")
    srl_in = nc.dram_tensor("srl_in", [2, 128, K], fp32, kind="ExternalInput")
    bsg_in = nc.dram_tensor("bsg_in", [2, 128, MH], fp32, kind="ExternalInput")
    wdr_in = nc.dram_tensor("wdr_in", [2, 128, NP, MH, 2, 128], fp8, kind="ExternalInput")
    wsage_in = nc.dram_tensor("wsage_in", [2, 2, 2, 128, H], bf16, kind="ExternalInput")
    wres_in = nc.dram_tensor("wres_in", [2, 2, 128, H], bf16, kind="ExternalInput")
    bmr_in = nc.dram_tensor("bmr_in", [2, 128, MH], fp32, kind="ExternalInput")
    wout_in = nc.dram_tensor("wout_in", [2, 128, CO], bf16, kind="ExternalInput")
    bout_in = nc.dram_tensor("bout_in", [64, 1], fp32, kind="ExternalInput")

    out_fm = nc.dram_tensor("out_fm", [CO, VL], fp32, kind="ExternalOutput")

    x1_T = nc.dram_tensor("x1_T", [T, 128, MH, VL], bf16)
    x0_nm = nc.dram_tensor("x0_nm", [T, VL, CIN], bf16)
    x1_nm = nc.dram_tensor("x1_nm", [T, VL, H], bf16)
    x0_full = nc.dram_tensor("x0_full", [T, NCORES * VL, CIN], bf16,
                             addr_space="Shared")
    x1_full = nc.dram_tensor("x1_full", [T, NCORES * VL, H], bf16,
                             addr_space="Shared")

    rg = [list(range(NCORES))]

    with tile.TileContext(nc) as tc, contextlib.ExitStack() as ctx:
        wpool = ctx.enter_context(tc.tile_pool(name="weights", bufs=1))
        lpool = ctx.enter_context(tc.tile_pool(name="layerw", bufs=1))
        spool = ctx.enter_context(tc.tile_pool(name="state", bufs=1))
        gpool = ctx.enter_context(tc.tile_pool(name="gather", bufs=2))
        spool8 = ctx.enter_context(tc.tile_pool(name="stiles", bufs=1))
        xpool = ctx.enter_context(tc.tile_pool(name="xt", bufs=2))
        hpool = ctx.enter_context(tc.tile_pool(name="hb", bufs=2))
        rpool = ctx.enter_context(tc.tile_pool(name="rl", bufs=1))
        mpool = ctx.enter_context(tc.tile_pool(name="misc", bufs=1))
        npool = ctx.enter_context(tc.tile_pool(name="nm", bufs=2))
        pp_y = ctx.enter_context(tc.tile_pool(name="py", bufs=1, space="PSUM"))
        pp_h = ctx.enter_context(tc.tile_pool(name="ph", bufs=1, space="PSUM"))
        pp_a = ctx.enter_context(tc.tile_pool(name="pa", bufs=1, space="PSUM"))

        # ---- persistent small weights
        wpre_d = wpool.tile([CIN, 3, CIN], bf16, tag="wpred")
        for tap in range(3):
            nc.sync.dma_start(out=wpre_d[:, tap, :], in_=wpre_diag_in[tap])
        bpre = wpool.tile([CIN, 1], fp32, tag="bpre")
        nc.sync.dma_start(out=bpre[:], in_=bpre_in[:])
        ident = wpool.tile([128, 128], bf16, tag="ident")
        nc.sync.dma_start(out=ident[:], in_=ident_in[:])
        lam_t = wpool.tile([128, 2, K], fp32, tag="lamt")
        srl_t = wpool.tile([128, 2, K], fp32, tag="srlt")
        bsg_t = wpool.tile([128, 2, MH], fp32, tag="bsgt")
        for L in range(2):
            nc.sync.dma_start(out=lam_t[:, L, :], in_=lam_in[L])
            nc.sync.dma_start(out=srl_t[:, L, :], in_=srl_in[L])
            nc.sync.dma_start(out=bsg_t[:, L, :], in_=bsg_in[L])
        wsage = wpool.tile([128, 2, 2, 2, H], bf16, tag="wsage")
        for L in range(2):
            for sn in range(2):
                for cc in range(2):
                    nc.sync.dma_start(out=wsage[:, L, sn, cc, :],
                                      in_=wsage_in[L, sn, cc])
        wres = wpool.tile([128, 2, 2, H], bf16, tag="wres")
        for L in range(2):
            for cc in range(2):
                nc.sync.dma_start(out=wres[:, L, cc, :], in_=wres_in[L, cc])
        bmr = wpool.tile([128, 2, MH], fp32, tag="bmr")
        for L in range(2):
            nc.sync.dma_start(out=bmr[:, L, :], in_=bmr_in[L])
        wout = wpool.tile([128, 2, CO], bf16, tag="wout")
        for cc in range(2):
            nc.sync.dma_start(out=wout[:, cc, :], in_=wout_in[cc])
        bout = wpool.tile([64, 1], fp32, tag="bout")
        nc.sync.dma_start(out=bout[:], in_=bout_in[:])

        # ---- x0 resident (feature-major, all t)
        x0sb = wpool.tile([128, T, VL], bf16, tag="x0sb")

        # ---- state; its flat bf16 view doubles as phase-A xs staging
        state = spool.tile([128, NP, 2 * VL], bf16, tag="state")
        sflat = state[:].rearrange("p a b -> p (a b)")

        # ---- phase A: token mix into x0sb, stage nm tables, AllGather per t
        for t in range(T):
            nc.sync.dma_start(out=sflat[:, t * VL:(t + 1) * VL], in_=xs_in[t])
        for t in range(T):
            sl = lambda u: sflat[:, u * VL:(u + 1) * VL]
            for (woff, wlen) in cfg.VCS:
                tm_ps = pp_y.tile([128, 512], fp32, tag="ya0", name="tmps")
                taps = [(tap, t + tap - 1) for tap in range(3)
                        if 0 <= t + tap - 1 < T]
                for i, (tap, u) in enumerate(taps):
                    nc.tensor.matmul(
                        out=tm_ps[:CIN, :wlen], lhsT=wpre_d[:, tap, :],
                        rhs=sl(u)[:, woff:woff + wlen],
                        start=(i == 0), stop=(i == len(taps) - 1))
                nc.scalar.activation(x0sb[:, t, woff:woff + wlen],
                                     tm_ps[:CIN, :wlen], AT.Identity,
                                     bias=bpre[:, 0:1], scale=1.0)
            nm = npool.tile([128, VL // 128, CIN], bf16, tag="nm")
            for bi in range(VL // 128):
                pt = pp_h.tile([128, 512], bf16, tag="h0", name="pt")
                nc.tensor.transpose(out=pt[:, :128],
                                    in_=x0sb[:, t, bi * 128:(bi + 1) * 128],
                                    identity=ident[:])
                nc.vector.tensor_copy(nm[:, bi, :CIN], pt[:, :CIN])
            nc.sync.dma_start(out=x0_nm[t].rearrange("(b p) c -> p b c", p=128),
                              in_=nm[:])
            if sim1:
                nc.sync.dma_start(out=x0_full[t, :VL, :], in_=x0_nm[t][:])
            else:
                nc.gpsimd.collective_compute(
                    "AllGather", OP.bypass, replica_groups=rg,
                    ins=[x0_nm[t][:]], outs=[x0_full[t][:]])

        def scan_layer(L):
            CC = cfg.CC[L]
            Cin = CIN if L == 0 else H
            xfull = x0_full if L == 0 else x1_full
            wdr = lpool.tile([128, NP, MH, 2, 128], fp8, tag="wdr")
            nc.sync.dma_start(out=wdr[:], in_=wdr_in[L])

            def prepare(t):
                # xt: feature-major input for res/self matmuls
                if L == 0:
                    xt = None  # use x0sb resident
                else:
                    xt = xpool.tile([128, MH, VL], bf16, tag="xin", name="xt")
                    nc.sync.dma_start(out=xt[:], in_=x1_T[t])
                xsl = (lambda cc: x0sb[:, t, :]) if L == 0 else \
                      (lambda cc: xt[:, cc, :])
                # gather + one-hot aggregation
                idx = gpool.tile([128, G * 64], i16, tag="idx", name="idx")
                nc.sync.dma_start(out=idx[:], in_=idx_in[t])
                s8 = spool8.tile([128, nchunk * 64], fp8, tag="soh", name="s8")
                nc.sync.dma_start(out=s8[:], in_=s8_in[t])
                ivd = gpool.tile([128, VL], bf16, tag="ivd", name="ivd")
                nc.sync.dma_start(out=ivd[:], in_=ivd_in[t])
                mean_sb = xpool.tile([128, 2, VL], bf16, tag="mean", name="mean_sb")
                win_of = {}
                for (woff, wlen) in cfg.VCS:
                    for b0 in range(woff // 64, (woff + wlen) // 64):
                        win_of[b0] = (woff, wlen)
                gt = None
                agg_ps = {}
                for ch in range(nchunk):
                    g, cg = divmod(ch, 8)
                    if cg == 0:
                        gt = gpool.tile([128, 8, Cin], bf16, tag="g", name="gt")
                        nc.gpsimd.dma_gather(
                            out_ap=gt[:], in_ap=xfull[t][:],
                            idxs_ap=idx[:, g * 64:(g + 1) * 64],
                            num_idxs=1024, num_idxs_reg=1024,
                            elem_size=Cin)
                    b, cb = divmod(ch, cfg.ncb)
                    woff, wlen = win_of[b]
                    if b % 8 == 0 and cb == 0:
                        agg_ps[0] = pp_a.tile([128, 512], fp32,
                                              tag="agg0", name="agg0")
                        if CC > 1:
                            # share the ya1 PSUM banks: L1 emits y only at
                            # t=7, after its prepare's aggs are done
                            agg_ps[1] = pp_y.tile([128, 512], fp32,
                                                  tag="ya1", name="agg1")
                    boff = b * 64 - woff
                    for cc in range(CC):
                        nc.tensor.matmul(
                            out=agg_ps[cc][:, boff:boff + 64],
                            lhsT=gt[:, cg, cc * 128:(cc + 1) * 128],
                            rhs=s8[:, ch * 64:(ch + 1) * 64], start=(cb == 0),
                            stop=(cb == cfg.ncb - 1))
                    if b == (woff + wlen) // 64 - 1 and cb == cfg.ncb - 1:
                        for cc in range(CC):
                            nc.scalar.activation(
                                mean_sb[:, cc, woff:woff + wlen],
                                agg_ps[cc][:, :wlen], AT.Copy)
                # mean *= invdeg (broadcast across cc)
                ms = mean_sb[:, :CC, :]
                nc.vector.tensor_tensor(
                    out=ms, in0=ms,
                    in1=ivd[:].unsqueeze(1).broadcast_to([128, CC, VL]),
                    op=OP.mult)
                # h = x@w_self + mean@w_neigh + bsg   -> h_sb [128, 2*VL]
                h_sb = hpool.tile([128, 2 * VL], bf16, tag="hsb", name="h_sb")
                for m in range(MH):
                    for (woff, wlen) in cfg.VCS:
                        h_ps = pp_h.tile([128, 512], fp32, tag="h0", name="h_ps")
                        for cc in range(CC):
                            nc.tensor.matmul(
                                out=h_ps[:, :wlen],
                                lhsT=wsage[:, L, 0, cc, m * 128:(m + 1) * 128],
                                rhs=xsl(cc)[:, woff:woff + wlen] if L == 0 else
                                    xt[:, cc, woff:woff + wlen],
                                start=(cc == 0), stop=False)
                        for cc in range(CC):
                            nc.tensor.matmul(
                                out=h_ps[:, :wlen],
                                lhsT=wsage[:, L, 1, cc, m * 128:(m + 1) * 128],
                                rhs=mean_sb[:, cc, woff:woff + wlen],
                                start=False, stop=(cc == CC - 1))
                        nc.scalar.activation(
                            h_sb[:, m * VL + woff:m * VL + woff + wlen],
                            h_ps[:, :wlen], AT.Identity,
                            bias=bsg_t[:, L, m:m + 1], scale=1.0)
                return xt, h_sb

            def state_k(k):
                j2, m = divmod(k, 2)
                return state[:, j2, m * VL:(m + 1) * VL]

            pre = prepare(0)
            h_prev = None
            for t in range(T):
                emit_y = (L == 0) or (t == T - 1)
                xt, h_sb = pre
                nxt = prepare(t + 1) if t + 1 < T else None

                # ---- state update + relu + DR mixes, pipelined over
                # pair-groups (t=0 is implicit: state_0 = h_0)
                if emit_y:
                    yA = [pp_y.tile([128, 1024], fp32, tag=f"ya{m}", name=f"ya{m}")
                          for m in range(MH)]
                    yB = [pp_y.tile([128, 256], fp32, tag=f"yb{m}", name=f"yb{m}")
                          for m in range(MH)]
                    def ypsw(m, woff, wlen):
                        if woff + wlen <= 1024:
                            return yA[m][:, woff:woff + wlen]
                        return yB[m][:, :wlen]
                    for m in range(MH):
                        for (woff, wlen) in cfg.VCS:
                            for cc in range(CC):
                                nc.tensor.matmul(
                                    out=ypsw(m, woff, wlen),
                                    lhsT=wres[:, L, cc, m * 128:(m + 1) * 128],
                                    rhs=(x0sb[:, t, woff:woff + wlen] if L == 0
                                         else xt[:, cc, woff:woff + wlen]),
                                    start=(cc == 0), stop=False)
                PG = NP // cfg.NGRP
                for g in range(cfg.NGRP):
                    j2s = range(g * PG, (g + 1) * PG)
                    if t == 1:
                        for j2 in j2s:
                            for ko in range(2):
                                k = 2 * j2 + ko
                                nc.vector.tensor_scalar(
                                    state_k(k),
                                    h_prev[:, ko * VL:(ko + 1) * VL],
                                    lam_t[:, L, k:k + 1], None, OP.mult)
                    elif t > 1:
                        for j2 in j2s:
                            for ko in range(2):
                                k = 2 * j2 + ko
                                if L == 1 and (k % 8) < cfg.TS1_ACT:
                                    nc.scalar.activation(
                                        state_k(k), state_k(k), AT.Copy,
                                        scale=lam_t[:, L, k:k + 1])
                                else:
                                    nc.vector.tensor_scalar(
                                        state_k(k), state_k(k),
                                        lam_t[:, L, k:k + 1], None, OP.mult)
                    if t > 0:
                        st3 = state[:, g * PG:(g + 1) * PG, :]
                        nc.vector.tensor_tensor(
                            out=st3, in0=st3,
                            in1=h_sb[:].unsqueeze(1)
                                .broadcast_to([128, PG, 2 * VL]),
                            op=OP.add)
                    if not emit_y:
                        continue
                    for j2 in j2s:
                        rl = rpool.tile([128, 2, VL], fp8, tag=f"rl{j2 % 3}",
                                        name="rl")
                        for ko in range(2):
                            k = 2 * j2 + ko
                            src = (h_sb[:, ko * VL:(ko + 1) * VL] if t == 0
                                   else state_k(k))
                            kr = k % 8
                            if kr < cfg.R_DVE:
                                nc.vector.tensor_scalar(
                                    rl[:, ko, :], src, srl_t[:, L, k:k + 1],
                                    0.0, OP.mult, OP.max)
                            elif kr < cfg.R_DVE + cfg.R_POOL:
                                nc.gpsimd.tensor_scalar(
                                    rl[:, ko, :], src, srl_t[:, L, k:k + 1],
                                    0.0, OP.mult, OP.max)
                            else:
                                nc.scalar.activation(
                                    rl[:, ko, :], src, AT.Relu,
                                    scale=srl_t[:, L, k:k + 1])
                        for m in range(MH):
                            for (woff, wlen) in cfg.VCS:
                                nc.tensor.matmul(
                                    out=ypsw(m, woff, wlen),
                                    lhsT=wdr[:, j2, m], perf_mode=DR,
                                    rhs=rl[:, :, woff:woff + wlen],
                                    start=False, stop=(j2 == NP - 1))
                if emit_y:
                    ys = mpool.tile([128, MH, VL], bf16, tag="ys")
                    for m in range(MH):
                        nc.scalar.activation(
                            ys[:, m, :1024], yA[m][:], AT.Identity,
                            bias=bmr[:, L, m:m + 1], scale=YDESCALE)
                        nc.scalar.activation(
                            ys[:, m, 1024:], yB[m][:, :VL - 1024], AT.Identity,
                            bias=bmr[:, L, m:m + 1], scale=YDESCALE)

                if L == 0:
                    nc.sync.dma_start(out=x1_T[t], in_=ys[:])
                    nm = npool.tile([128, VL // 128, H], bf16, tag="nm")
                    for bi in range(VL // 128):
                        for m in range(MH):
                            pt = pp_y.tile([128, 256], bf16, tag="yb0", name="pt")
                            nc.tensor.transpose(
                                out=pt[:, :128], in_=ys[:, m, bi * 128:(bi + 1) * 128],
                                identity=ident[:])
                            nc.vector.tensor_copy(
                                nm[:, bi, m * 128:(m + 1) * 128], pt[:, :128])
                    nc.sync.dma_start(
                        out=x1_nm[t].rearrange("(b p) c -> p b c", p=128),
                        in_=nm[:])
                    if sim1:
                        nc.sync.dma_start(out=x1_full[t, :VL, :], in_=x1_nm[t][:])
                    else:
                        nc.gpsimd.collective_compute(
                            "AllGather", OP.bypass, replica_groups=rg,
                            ins=[x1_nm[t][:]], outs=[x1_full[t][:]])
                if L == 1 and emit_y:
                    ot = mpool.tile([CO, VL], fp32, tag="outt")
                    for (woff, wlen) in cfg.VCS:
                        o_ps = pp_h.tile([CO, 512], fp32, tag="h0", name="o_ps")
                        for cc in range(MH):
                            nc.tensor.matmul(
                                out=o_ps[:, :wlen], lhsT=wout[:, cc, :],
                                rhs=ys[:, cc, woff:woff + wlen],
                                start=(cc == 0), stop=(cc == MH - 1))
                        nc.scalar.activation(ot[:, woff:woff + wlen],
                                             o_ps[:, :wlen], AT.Identity,
                                             bias=bout[:, 0:1], scale=1.0)
                    nc.sync.dma_start(out=out_fm[:], in_=ot[:])

                h_prev = h_sb
                pre = nxt
        scan_layer(0)
        scan_layer(1)

    nc.compile()
    return nc


# ----------------------------------------------------------------------------
# host wrapper
# ----------------------------------------------------------------------------

def make_inputs(cfg, inputs):
    T, CIN, H, DS, CO = cfg.T, cfg.CIN, cfg.H, cfg.DS, cfg.CO
    VLOC, VL, K, NP, MH = cfg.VLOC, cfg.VL, cfg.K, cfg.NP, cfg.MH
    xs = np.asarray(inputs["xs"], np.float32)
    src_rows, scol, invdeg = prep_edges(cfg, inputs["edge_index"])
    nchunk = cfg.NB * cfg.ncb

    w_pre = np.asarray(inputs["w_pre"], np.float32)
    wpre_diag = np.stack([np.diag(w_pre[:, tap]) for tap in range(3)]).astype(BF16)
    bpre = np.asarray(inputs["b_pre"], np.float32).reshape(CIN, 1)
    ident = np.eye(128, dtype=np.float32).astype(BF16)

    lam_a, srl_a, bsg_a, wdr_a, wsage_a, wres_a, bmr_a = [], [], [], [], [], [], []
    for L, f in ((0, CIN), (1, H)):
        lam_ij = np.exp(-np.exp(np.asarray(inputs[f"a_log{L}"], np.float64))) \
            .astype(np.float32)                                   # [H, DS]
        B_ij = np.asarray(inputs[f"B{L}"], np.float32)            # [H, DS]
        wm = np.asarray(inputs[f"w_mix{L}"], np.float32)          # [H*DS, H]
        # k = 2*j2 + m ; row p of tile k holds (i = m*128+p, j = j2)
        lam_k = np.zeros((128, K), np.float32)
        srl_k = np.zeros((128, K), np.float32)
        wdr_l = np.zeros((128, NP, MH, 2, 128), np.float32)
        p = np.arange(128)
        for j2 in range(NP):
            for m in range(2):
                i = m * 128 + p
                k = 2 * j2 + m
                lam_k[:, k] = lam_ij[i, j2]
                srl_k[:, k] = np.sign(B_ij[i, j2]) * RLSCALE
        for j2 in range(NP):
            for mh in range(MH):
                for ko in range(2):
                    i = ko * 128 + p
                    w_rows = wm[i * DS + j2, mh * 128:(mh + 1) * 128]  # [128,128]
                    w_rows = w_rows * np.abs(B_ij[i, j2])[:, None] * WSCALE
                    wdr_l[:, j2, mh, ko, :] = w_rows
        lam_a.append(lam_k)
        srl_a.append(srl_k)
        wdr_a.append(wdr_l.astype(FP8))
        bsg_a.append(np.asarray(inputs[f"b_sage{L}"], np.float32)
                     .reshape(MH, 128).T)
        ws = np.zeros((2, 2, 128, H), np.float32)
        wsf = np.asarray(inputs[f"w_self{L}"], np.float32)
        wnf = np.asarray(inputs[f"w_neigh{L}"], np.float32)
        for cc in range((f + 127) // 128):
            ws[0, cc] = wsf[cc * 128:(cc + 1) * 128]
            ws[1, cc] = wnf[cc * 128:(cc + 1) * 128]
        wsage_a.append(ws.astype(BF16))
        wr = np.zeros((2, 128, H), np.float32)
        wrf = np.asarray(inputs[f"w_res{L}"], np.float32) * (RLSCALE * WSCALE)
        for cc in range((f + 127) // 128):
            wr[cc] = wrf[cc * 128:(cc + 1) * 128]
        wres_a.append(wr.astype(BF16))
        bmr_a.append((np.asarray(inputs[f"b_res{L}"], np.float32)
                      + np.asarray(inputs[f"b_mix{L}"], np.float32))
                     .reshape(MH, 128).T)

    wout = np.asarray(inputs["w_out"], np.float32).reshape(MH, 128, CO).astype(BF16)
    bout = np.asarray(inputs["b_out"], np.float32).reshape(CO, 1)
    wcol = np.arange(64)

    in_maps = []
    for c in range(NCORES):
        onehot = (scol[c][..., None] == wcol)                 # [T, nchunk, 128, 64]
        s8 = np.ascontiguousarray(
            onehot.transpose(0, 2, 1, 3).reshape(T, 128, nchunk * 64)
        ).astype(FP8)
        sh = xs[:, c * VLOC:(c + 1) * VLOC, :]
        xs_sh = np.zeros((T, CIN, VL), np.float32)
        xs_sh[:, :, :VLOC] = np.transpose(sh, (0, 2, 1))
        ivd = np.broadcast_to(invdeg[:, c][:, None, :], (T, 128, VL))
        in_maps.append({
            "xs_in": xs_sh.astype(BF16),
            "idx_in": pack_gather_idx(cfg, src_rows[c]),
            "s8_in": s8,
            "ivd_in": np.ascontiguousarray(ivd).astype(BF16),
            "wpre_diag_in": wpre_diag,
            "bpre_in": bpre,
            "ident_in": ident,
            "lam_in": np.stack(lam_a),
            "srl_in": np.stack(srl_a),
            "bsg_in": np.stack(bsg_a).astype(np.float32),
            "wdr_in": np.stack(wdr_a),
            "wsage_in": np.stack(wsage_a),
            "wres_in": np.stack(wres_a),
            "bmr_in": np.stack(bmr_a).astype(np.float32),
            "wout_in": wout,
            "bout_in": bout,
        })
    return in_maps


_CACHED = {}


def kernel(**inputs):
    cfg = Cfg()
    in_maps = make_inputs(cfg, inputs)
    key = ("full", cfg.ncb)
    if key not in _CACHED:
        _CACHED[key] = build_program(cfg)
    nc = _CACHED[key]
    res = run_bass_kernel_spmd(nc, in_maps, list(range(NCORES)))
    out = np.zeros((cfg.V, cfg.CO), np.float32)
    for c in range(NCORES):
        out[c * cfg.VLOC:(c + 1) * cfg.VLOC] = \
            res.results[c]["out_fm"][:, :cfg.VLOC].T
    return out


# revision 9
# speedup vs baseline: 1.3250x; 1.1244x over previous
"""Trainium2 Bass kernel for nn_DiagonalSSM (token-mix -> 2x [SAGE + diagonal SSM scan] -> proj).

Sharding: nodes (V) split across 8 cores; per-core dst-sorted edge chunks with
one-hot fp8 S tiles drive the segment-mean as PE matmuls on dma-gathered
source rows; inverse-degree applied afterwards on the mean via a broadcast
tensor_tensor multiply.

SSM state [128, 16 pairs, 2*VL] bf16 ordered k=(j2, m) so the per-step h add
is ONE broadcast tensor_tensor over the whole state (h appears once per pair).
The lam multiply is 32 per-tile tensor_scalars (lam varies per (k,p)). relu
emits fp8 tiles consumed by fp8 DoubleRow matmuls (2 k-tiles of contraction
per instruction, 0.5 cyc/row) with power-of-two scaling: wmix*128, rl*8,
descale 2^-10 folded into the PSUM->SBUF copy; w_res is scaled *1024 in bf16
so residual and mix share one PSUM accumulation group.
"""

import contextlib

import numpy as np
import ml_dtypes

import concourse.bacc as bacc
import concourse.bass as bass
import concourse.mybir as mybir
import concourse.tile as tile
from concourse.bass_utils import run_bass_kernel_spmd

BF16 = ml_dtypes.bfloat16
FP8 = ml_dtypes.float8_e4m3

NCORES = 8
RLSCALE = 8.0
WSCALE = 128.0
YDESCALE = 1.0 / (RLSCALE * WSCALE)


class Cfg:
    def __init__(self, T=8, V=10000, E=100000, CIN=128, H=256, DS=16, CO=64):
        self.T, self.V, self.E = T, V, E
        self.CIN, self.H, self.DS, self.CO = CIN, H, DS, CO
        self.VLOC = V // NCORES                      # real nodes per core
        self.VL = ((self.VLOC + 127) // 128) * 128   # padded local nodes
        self.NB = self.VL // 64                      # 64-node dst blocks
        self.ncb = None                              # chunks per block (set by prep)
        self.VCS = []                                # v-chunk windows (<=512)
        off = 0
        while off < self.VL:
            w = min(512, self.VL - off)
            self.VCS.append((off, w))
            off += w
        self.K = (H * DS) // 128                     # state tiles
        self.NP = self.K // 2                        # (j2) pairs
        self.MH = H // 128                           # output chunks of H
        self.CC = {0: max(1, CIN // 128), 1: H // 128}
        # engine split knobs
        self.R_DVE = 1      # relus per 8 k-tiles on DVE
        self.R_POOL = 2     # relus per 8 k-tiles on GPSIMD (rest on Act)
        self.TS1_ACT = 0    # L1 lam-mults per step on Act (rest DVE)
        self.NGRP = 4       # pair-groups per state update (pipelining)


# ----------------------------------------------------------------------------
# host-side preparation
# ----------------------------------------------------------------------------

def prep_edges(cfg, edge_index):
    T = cfg.T
    VLOC, VL, NB = cfg.VLOC, cfg.VL, cfg.NB
    ei = np.asarray(edge_index)
    src_all, dst_all = ei[:, 0, :].astype(np.int64), ei[:, 1, :].astype(np.int64)

    buckets = [[[None] * NB for _ in range(T)] for _ in range(NCORES)]
    deg = np.zeros((T, NCORES, VL), np.float32)
    for t in range(T):
        s_t, d_t = src_all[t], dst_all[t]
        core = np.minimum(d_t // VLOC, NCORES - 1)
        for c in range(NCORES):
            m = core == c
            s_c, d_c = s_t[m], d_t[m] - c * VLOC
            np.add.at(deg[t, c], d_c, 1.0)
            b_c = d_c // 64
            order = np.argsort(b_c, kind="stable")
            s_c, d_c, b_c = s_c[order], d_c[order], b_c[order]
            bounds = np.searchsorted(b_c, np.arange(NB + 1))
            for b in range(NB):
                lo, hi = bounds[b], bounds[b + 1]
                buckets[c][t][b] = (s_c[lo:hi], d_c[lo:hi])

    ncb = 1
    for c in range(NCORES):
        for t in range(T):
            for b in range(NB):
                ncb = max(ncb, (len(buckets[c][t][b][0]) + 127) // 128)
    cfg.ncb = ncb
    nchunk = NB * ncb
    invdeg = 1.0 / np.maximum(deg, 1.0)

    src_rows = np.zeros((NCORES, T, nchunk, 128), np.int16)
    scol = np.full((NCORES, T, nchunk, 128), -1, np.int64)
    for c in range(NCORES):
        for t in range(T):
            for b in range(NB):
                s_b, d_b = buckets[c][t][b]
                n = len(s_b)
                pad = ncb * 128 - n
                rows = (s_b // VLOC) * VL + (s_b % VLOC)
                rows = np.concatenate([rows, np.zeros(pad, np.int64)])
                col = np.concatenate([d_b - b * 64, np.full(pad, -1, np.int64)])
                cs = b * ncb
                src_rows[c, t, cs:cs + ncb] = rows.reshape(ncb, 128).astype(np.int16)
                scol[c, t, cs:cs + ncb] = col.reshape(ncb, 128)
    return src_rows, scol, invdeg


def pack_gather_idx(cfg, src_rows):
    # [T, 128, G*64] int16; 1024 idxs per gather call = 16 partitions x 64
    # cols, replicated 8x across the 128 partitions.
    T = cfg.T
    nchunk = src_rows.shape[1]
    G = (nchunk + 7) // 8
    out = np.zeros((T, 128, G * 64), np.int16)
    for t in range(T):
        flat = np.zeros(G * 1024, np.int16)
        flat[:nchunk * 128] = src_rows[t].reshape(-1)
        out[t] = np.tile(flat.reshape(-1, 16).T, (8, 1))
    return out


# ----------------------------------------------------------------------------
# device program
# ----------------------------------------------------------------------------

def build_program(cfg, sim1=False):
    T, VL, CIN, H, DS, CO = cfg.T, cfg.VL, cfg.CIN, cfg.H, cfg.DS, cfg.CO
    K, NP, MH = cfg.K, cfg.NP, cfg.MH
    nchunk = cfg.NB * cfg.ncb
    G = (nchunk + 7) // 8
    fp32, bf16 = mybir.dt.float32, mybir.dt.bfloat16
    fp8, i16 = mybir.dt.float8e4, mybir.dt.int16
    AT = mybir.ActivationFunctionType
    OP = mybir.AluOpType
    DR = mybir.MatmulPerfMode.DoubleRow

    ndev = 1 if sim1 else NCORES
    nc = bacc.Bacc("TRN2", target_bir_lowering=False, debug=False,
                   num_devices=ndev)

    xs_in = nc.dram_tensor("xs_in", [T, CIN, VL], bf16, kind="ExternalInput")
    idx_in = nc.dram_tensor("idx_in", [T, 128, G * 64], i16, kind="ExternalInput")
    s8_in = nc.dram_tensor("s8_in", [T, 128, nchunk * 64], fp8, kind="ExternalInput")
    ivd_in = nc.dram_tensor("ivd_in", [T, 128, VL], bf16, kind="ExternalInput")
    wpre_diag_in = nc.dram_tensor("wpre_diag_in", [3, CIN, CIN], bf16, kind="ExternalInput")
    bpre_in = nc.dram_tensor("bpre_in", [CIN, 1], fp32, kind="ExternalInput")
    ident_in = nc.dram_tensor("ident_in", [128, 128], bf16, kind="ExternalInput")
    lam_in = nc.dram_tensor("lam_in", [2, 128, K], fp32, kind="ExternalInput")
    srl_in = nc.dram_tensor("srl_in", [2, 128, K], fp32, kind="ExternalInput")
    bsg_in = nc.dram_tensor("bsg_in", [2, 128, MH], fp32, kind="ExternalInput")
    wdr_in = nc.dram_tensor("wdr_in", [2, 128, NP, MH, 2, 128], fp8, kind="ExternalInput")
    wsage_in = nc.dram_tensor("wsage_in", [2, 2, 2, 128, H], bf16, kind="ExternalInput")
    wres_in = nc.dram_tensor("wres_in", [2, 2, 128, H], bf16, kind="ExternalInput")
    bmr_in = nc.dram_tensor("bmr_in", [2, 128, MH], fp32, kind="ExternalInput")
    wout_in = nc.dram_tensor("wout_in", [2, 128, CO], bf16, kind="ExternalInput")
    bout_in = nc.dram_tensor("bout_in", [64, 1], fp32, kind="ExternalInput")

    out_fm = nc.dram_tensor("out_fm", [CO, VL], fp32, kind="ExternalOutput")

    x0_T = nc.dram_tensor("x0_T", [T, 128, VL], bf16)
    x1_T = nc.dram_tensor("x1_T", [T, 128, MH, VL], bf16)
    x0_nm = nc.dram_tensor("x0_nm", [T, VL, CIN], bf16)
    x1_nm = nc.dram_tensor("x1_nm", [T, VL, H], bf16)
    x0_full = nc.dram_tensor("x0_full", [T, NCORES * VL, CIN], bf16,
                             addr_space="Shared")
    x1_full = nc.dram_tensor("x1_full", [T, NCORES * VL, H], bf16,
                             addr_space="Shared")

    rg = [list(range(NCORES))]

    with tile.TileContext(nc) as tc, contextlib.ExitStack() as ctx:
        wpool = ctx.enter_context(tc.tile_pool(name="weights", bufs=1))
        lpool = ctx.enter_context(tc.tile_pool(name="layerw", bufs=1))
        spool = ctx.enter_context(tc.tile_pool(name="state", bufs=1))
        gpool = ctx.enter_context(tc.tile_pool(name="gather", bufs=3))
        ipool = ctx.enter_context(tc.tile_pool(name="idx", bufs=2))
        spool8 = ctx.enter_context(tc.tile_pool(name="stiles", bufs=2))
        xpool = ctx.enter_context(tc.tile_pool(name="xt", bufs=3))
        hpool = ctx.enter_context(tc.tile_pool(name="hb", bufs=3))
        rpool = ctx.enter_context(tc.tile_pool(name="rl", bufs=1))
        mpool = ctx.enter_context(tc.tile_pool(name="misc", bufs=1))
        npool = ctx.enter_context(tc.tile_pool(name="nm", bufs=1))
        pp_y = ctx.enter_context(tc.tile_pool(name="py", bufs=1, space="PSUM"))
        pp_h = ctx.enter_context(tc.tile_pool(name="ph", bufs=1, space="PSUM"))
        pp_a = ctx.enter_context(tc.tile_pool(name="pa", bufs=1, space="PSUM"))

        # ---- persistent small weights
        wpre_d = wpool.tile([CIN, 3, CIN], bf16, tag="wpred")
        for tap in range(3):
            nc.sync.dma_start(out=wpre_d[:, tap, :], in_=wpre_diag_in[tap])
        bpre = wpool.tile([CIN, 1], fp32, tag="bpre")
        nc.sync.dma_start(out=bpre[:], in_=bpre_in[:])
        ident = wpool.tile([128, 128], bf16, tag="ident")
        nc.sync.dma_start(out=ident[:], in_=ident_in[:])
        lam_t = wpool.tile([128, 2, K], fp32, tag="lamt")
        srl_t = wpool.tile([128, 2, K], fp32, tag="srlt")
        bsg_t = wpool.tile([128, 2, MH], fp32, tag="bsgt")
        for L in range(2):
            nc.sync.dma_start(out=lam_t[:, L, :], in_=lam_in[L])
            nc.sync.dma_start(out=srl_t[:, L, :], in_=srl_in[L])
            nc.sync.dma_start(out=bsg_t[:, L, :], in_=bsg_in[L])
        wsage = wpool.tile([128, 2, 2, 2, H], bf16, tag="wsage")
        for L in range(2):
            for sn in range(2):
                for cc in range(2):
                    nc.sync.dma_start(out=wsage[:, L, sn, cc, :],
                                      in_=wsage_in[L, sn, cc])
        wres = wpool.tile([128, 2, 2, H], bf16, tag="wres")
        for L in range(2):
            for cc in range(2):
                nc.sync.dma_start(out=wres[:, L, cc, :], in_=wres_in[L, cc])
        bmr = wpool.tile([128, 2, MH], fp32, tag="bmr")
        for L in range(2):
            nc.sync.dma_start(out=bmr[:, L, :], in_=bmr_in[L])
        wout = wpool.tile([128, 2, CO], bf16, tag="wout")
        for cc in range(2):
            nc.sync.dma_start(out=wout[:, cc, :], in_=wout_in[cc])
        bout = wpool.tile([64, 1], fp32, tag="bout")
        nc.sync.dma_start(out=bout[:], in_=bout_in[:])

        # ---- state; its flat bf16 view doubles as phase-A xs staging
        state = spool.tile([128, NP, 2 * VL], bf16, tag="state")
        sflat = state[:].rearrange("p a b -> p (a b)")

        # ---- phase A: token mix into x0sb, stage nm tables, AllGather per t
        for t in range(T):
            nc.sync.dma_start(out=sflat[:, t * VL:(t + 1) * VL], in_=xs_in[t])
        for t in range(T):
            sl = lambda u: sflat[:, u * VL:(u + 1) * VL]
            x0t = mpool.tile([CIN, VL], bf16, tag="x0t")
            for (woff, wlen) in cfg.VCS:
                tm_ps = pp_y.tile([128, 512], fp32, tag="ya0", name="tmps")
                taps = [(tap, t + tap - 1) for tap in range(3)
                        if 0 <= t + tap - 1 < T]
                for i, (tap, u) in enumerate(taps):
                    nc.tensor.matmul(
                        out=tm_ps[:CIN, :wlen], lhsT=wpre_d[:, tap, :],
                        rhs=sl(u)[:, woff:woff + wlen],
                        start=(i == 0), stop=(i == len(taps) - 1))
                nc.scalar.activation(x0t[:, woff:woff + wlen],
                                     tm_ps[:CIN, :wlen], AT.Identity,
                                     bias=bpre[:, 0:1], scale=1.0)
            nc.sync.dma_start(out=x0_T[t], in_=x0t[:])
            nm = npool.tile([128, VL // 128, CIN], bf16, tag="nm")
            for bi in range(VL // 128):
                pt = pp_h.tile([128, 512], bf16, tag="h0", name="pt")
                nc.tensor.transpose(out=pt[:, :128],
                                    in_=x0t[:, bi * 128:(bi + 1) * 128],
                                    identity=ident[:])
                nc.vector.tensor_copy(nm[:, bi, :CIN], pt[:, :CIN])
            nc.sync.dma_start(out=x0_nm[t].rearrange("(b p) c -> p b c", p=128),
                              in_=nm[:])
            if sim1:
                nc.sync.dma_start(out=x0_full[t, :VL, :], in_=x0_nm[t][:])
            else:
                nc.gpsimd.collective_compute(
                    "AllGather", OP.bypass, replica_groups=rg,
                    ins=[x0_nm[t][:]], outs=[x0_full[t][:]])

        def scan_layer(L):
            CC = cfg.CC[L]
            Cin = CIN if L == 0 else H
            xfull = x0_full if L == 0 else x1_full
            wdr = lpool.tile([128, NP, MH, 2, 128], fp8, tag="wdr")
            nc.sync.dma_start(out=wdr[:], in_=wdr_in[L])

            def prepare(t):
                # xt: feature-major input for res/self matmuls
                xt = xpool.tile([128, MH, VL], bf16, tag="xin", name="xt")
                if L == 0:
                    nc.sync.dma_start(out=xt[:, 0, :], in_=x0_T[t])
                else:
                    nc.sync.dma_start(out=xt[:], in_=x1_T[t])
                # gather + one-hot aggregation; h assembled per 512-window
                idx = ipool.tile([128, G * 64], i16, tag="idx", name="idx")
                nc.sync.dma_start(out=idx[:], in_=idx_in[t])
                s8 = spool8.tile([128, nchunk * 64], fp8, tag="soh", name="s8")
                nc.sync.dma_start(out=s8[:], in_=s8_in[t])
                ivd = ipool.tile([128, VL], bf16, tag="ivd", name="ivd")
                nc.sync.dma_start(out=ivd[:], in_=ivd_in[t])
                mean_sb = xpool.tile([128, 2, VL], bf16, tag="mean", name="mean_sb")
                h_sb = hpool.tile([128, 2 * VL], bf16, tag="hsb", name="h_sb")
                win_of = {}
                for (woff, wlen) in cfg.VCS:
                    for b0 in range(woff // 64, (woff + wlen) // 64):
                        win_of[b0] = (woff, wlen)

                def finish_window(woff, wlen):
                    # mean(w) = agg(w) * invdeg(w); h(m, w) right behind
                    for cc in range(CC):
                        nc.scalar.activation(
                            mean_sb[:, cc, woff:woff + wlen],
                            agg_ps[cc][:, :wlen], AT.Copy)
                    ms = mean_sb[:, :CC, woff:woff + wlen]
                    nc.vector.tensor_tensor(
                        out=ms, in0=ms,
                        in1=ivd[:, woff:woff + wlen].unsqueeze(1)
                            .broadcast_to([128, CC, wlen]),
                        op=OP.mult)
                    for m in range(MH):
                        h_ps = pp_h.tile([128, 512], fp32, tag="h0", name="h_ps")
                        for cc in range(CC):
                            nc.tensor.matmul(
                                out=h_ps[:, :wlen],
                                lhsT=wsage[:, L, 0, cc, m * 128:(m + 1) * 128],
                                rhs=xt[:, cc if L else 0, woff:woff + wlen],
                                start=(cc == 0), stop=False)
                        for cc in range(CC):
                            nc.tensor.matmul(
                                out=h_ps[:, :wlen],
                                lhsT=wsage[:, L, 1, cc, m * 128:(m + 1) * 128],
                                rhs=mean_sb[:, cc, woff:woff + wlen],
                                start=False, stop=(cc == CC - 1))
                        nc.scalar.activation(
                            h_sb[:, m * VL + woff:m * VL + woff + wlen],
                            h_ps[:, :wlen], AT.Identity,
                            bias=bsg_t[:, L, m:m + 1], scale=1.0)

                gt = None
                agg_ps = {}
                for ch in range(nchunk):
                    g, cg = divmod(ch, 8)
                    if cg == 0:
                        gt = gpool.tile([128, 8, Cin], bf16, tag="g", name="gt")
                        nc.gpsimd.dma_gather(
                            out_ap=gt[:], in_ap=xfull[t][:],
                            idxs_ap=idx[:, g * 64:(g + 1) * 64],
                            num_idxs=1024, num_idxs_reg=1024,
                            elem_size=Cin)
                    b, cb = divmod(ch, cfg.ncb)
                    woff, wlen = win_of[b]
                    if b % 8 == 0 and cb == 0:
                        agg_ps[0] = pp_a.tile([128, 512], fp32,
                                              tag="agg0", name="agg0")
                        if CC > 1:
                            # share the ya1 PSUM banks: L1 emits y only at
                            # t=7, after its prepare's aggs are done
                            agg_ps[1] = pp_y.tile([128, 512], fp32,
                                                  tag="ya1", name="agg1")
                    boff = b * 64 - woff
                    for cc in range(CC):
                        nc.tensor.matmul(
                            out=agg_ps[cc][:, boff:boff + 64],
                            lhsT=gt[:, cg, cc * 128:(cc + 1) * 128],
                            rhs=s8[:, ch * 64:(ch + 1) * 64], start=(cb == 0),
                            stop=(cb == cfg.ncb - 1))
                    if b == (woff + wlen) // 64 - 1 and cb == cfg.ncb - 1:
                        finish_window(woff, wlen)
                return xt, h_sb

            def state_k(k):
                j2, m = divmod(k, 2)
                return state[:, j2, m * VL:(m + 1) * VL]

            pres = {0: prepare(0), 1: prepare(1)}
            h_prev = None
            for t in range(T):
                emit_y = (L == 0) or (t == T - 1)
                xt, h_sb = pres.pop(t)
                if t + 2 < T:
                    pres[t + 2] = prepare(t + 2)

                # ---- state update + relu + DR mixes, pipelined over
                # pair-groups (t=0 is implicit: state_0 = h_0)
                if emit_y:
                    yA = [pp_y.tile([128, 1024], fp32, tag=f"ya{m}", name=f"ya{m}")
                          for m in range(MH)]
                    yB = [pp_y.tile([128, 256], fp32, tag=f"yb{m}", name=f"yb{m}")
                          for m in range(MH)]
                    def ypsw(m, woff, wlen):
                        if woff + wlen <= 1024:
                            return yA[m][:, woff:woff + wlen]
                        return yB[m][:, :wlen]
                    for m in range(MH):
                        for (woff, wlen) in cfg.VCS:
                            for cc in range(CC):
                                nc.tensor.matmul(
                                    out=ypsw(m, woff, wlen),
                                    lhsT=wres[:, L, cc, m * 128:(m + 1) * 128],
                                    rhs=xt[:, cc if L else 0,
                                           woff:woff + wlen],
                                    start=(cc == 0), stop=False)
                PG = NP // cfg.NGRP
                for g in range(cfg.NGRP):
                    j2s = range(g * PG, (g + 1) * PG)
                    if t == 1:
                        for j2 in j2s:
                            for ko in range(2):
                                k = 2 * j2 + ko
                                nc.vector.tensor_scalar(
                                    state_k(k),
                                    h_prev[:, ko * VL:(ko + 1) * VL],
                                    lam_t[:, L, k:k + 1], None, OP.mult)
                    elif t > 1:
                        for j2 in j2s:
                            for ko in range(2):
                                k = 2 * j2 + ko
                                if L == 1 and (k % 8) < cfg.TS1_ACT:
                                    nc.scalar.activation(
                                        state_k(k), state_k(k), AT.Copy,
                                        scale=lam_t[:, L, k:k + 1])
                                else:
                                    nc.vector.tensor_scalar(
                                        state_k(k), state_k(k),
                                        lam_t[:, L, k:k + 1], None, OP.mult)
                    if t > 0:
                        st3 = state[:, g * PG:(g + 1) * PG, :]
                        nc.vector.tensor_tensor(
                            out=st3, in0=st3,
                            in1=h_sb[:].unsqueeze(1)
                                .broadcast_to([128, PG, 2 * VL]),
                            op=OP.add)
                    if not emit_y:
                        continue
                    for j2 in j2s:
                        rl = rpool.tile([128, 2, VL], fp8, tag=f"rl{j2 % 3}",
                                        name="rl")
                        for ko in range(2):
                            k = 2 * j2 + ko
                            src = (h_sb[:, ko * VL:(ko + 1) * VL] if t == 0
                                   else state_k(k))
                            kr = k % 8
                            if kr < cfg.R_DVE:
                                nc.vector.tensor_scalar(
                                    rl[:, ko, :], src, srl_t[:, L, k:k + 1],
                                    0.0, OP.mult, OP.max)
                            elif kr < cfg.R_DVE + cfg.R_POOL:
                                nc.gpsimd.tensor_scalar(
                                    rl[:, ko, :], src, srl_t[:, L, k:k + 1],
                                    0.0, OP.mult, OP.max)
                            else:
                                nc.scalar.activation(
                                    rl[:, ko, :], src, AT.Relu,
                                    scale=srl_t[:, L, k:k + 1])
                        for m in range(MH):
                            for (woff, wlen) in cfg.VCS:
                                nc.tensor.matmul(
                                    out=ypsw(m, woff, wlen),
                                    lhsT=wdr[:, j2, m], perf_mode=DR,
                                    rhs=rl[:, :, woff:woff + wlen],
                                    start=False, stop=(j2 == NP - 1))
                if emit_y:
                    ys = mpool.tile([128, MH, VL], bf16, tag="ys")
                    for m in range(MH):
                        nc.scalar.activation(
                            ys[:, m, :1024], yA[m][:], AT.Identity,
                            bias=bmr[:, L, m:m + 1], scale=YDESCALE)
                        nc.scalar.activation(
                            ys[:, m, 1024:], yB[m][:, :VL - 1024], AT.Identity,
                            bias=bmr[:, L, m:m + 1], scale=YDESCALE)

                if L == 0:
                    nc.sync.dma_start(out=x1_T[t], in_=ys[:])
                    nm = npool.tile([128, VL // 128, H], bf16, tag="nm")
                    for bi in range(VL // 128):
                        for m in range(MH):
                            pt = pp_y.tile([128, 256], bf16, tag="yb0", name="pt")
                            nc.tensor.transpose(
                                out=pt[:, :128], in_=ys[:, m, bi * 128:(bi + 1) * 128],
                                identity=ident[:])
                            nc.vector.tensor_copy(
                                nm[:, bi, m * 128:(m + 1) * 128], pt[:, :128])
                    nc.sync.dma_start(
                        out=x1_nm[t].rearrange("(b p) c -> p b c", p=128),
                        in_=nm[:])
                    if sim1:
                        nc.sync.dma_start(out=x1_full[t, :VL, :], in_=x1_nm[t][:])
                    else:
                        nc.gpsimd.collective_compute(
                            "AllGather", OP.bypass, replica_groups=rg,
                            ins=[x1_nm[t][:]], outs=[x1_full[t][:]])
                if L == 1 and emit_y:
                    ot = mpool.tile([CO, VL], fp32, tag="outt")
                    for (woff, wlen) in cfg.VCS:
                        o_ps = pp_h.tile([CO, 512], fp32, tag="h0", name="o_ps")
                        for cc in range(MH):
                            nc.tensor.matmul(
                                out=o_ps[:, :wlen], lhsT=wout[:, cc, :],
                                rhs=ys[:, cc, woff:woff + wlen],
                                start=(cc == 0), stop=(cc == MH - 1))
                        nc.scalar.activation(ot[:, woff:woff + wlen],
                                             o_ps[:, :wlen], AT.Identity,
                                             bias=bout[:, 0:1], scale=1.0)
                    nc.sync.dma_start(out=out_fm[:], in_=ot[:])

                h_prev = h_sb
        scan_layer(0)
        scan_layer(1)

    nc.compile()
    return nc


# ----------------------------------------------------------------------------
# host wrapper
# ----------------------------------------------------------------------------

def make_inputs(cfg, inputs):
    T, CIN, H, DS, CO = cfg.T, cfg.CIN, cfg.H, cfg.DS, cfg.CO
    VLOC, VL, K, NP, MH = cfg.VLOC, cfg.VL, cfg.K, cfg.NP, cfg.MH
    xs = np.asarray(inputs["xs"], np.float32)
    src_rows, scol, invdeg = prep_edges(cfg, inputs["edge_index"])
    nchunk = cfg.NB * cfg.ncb

    w_pre = np.asarray(inputs["w_pre"], np.float32)
    wpre_diag = np.stack([np.diag(w_pre[:, tap]) for tap in range(3)]).astype(BF16)
    bpre = np.asarray(inputs["b_pre"], np.float32).reshape(CIN, 1)
    ident = np.eye(128, dtype=np.float32).astype(BF16)

    lam_a, srl_a, bsg_a, wdr_a, wsage_a, wres_a, bmr_a = [], [], [], [], [], [], []
    for L, f in ((0, CIN), (1, H)):
        lam_ij = np.exp(-np.exp(np.asarray(inputs[f"a_log{L}"], np.float64))) \
            .astype(np.float32)                                   # [H, DS]
        B_ij = np.asarray(inputs[f"B{L}"], np.float32)            # [H, DS]
        wm = np.asarray(inputs[f"w_mix{L}"], np.float32)          # [H*DS, H]
        # k = 2*j2 + m ; row p of tile k holds (i = m*128+p, j = j2)
        lam_k = np.zeros((128, K), np.float32)
        srl_k = np.zeros((128, K), np.float32)
        wdr_l = np.zeros((128, NP, MH, 2, 128), np.float32)
        p = np.arange(128)
        for j2 in range(NP):
            for m in range(2):
                i = m * 128 + p
                k = 2 * j2 + m
                lam_k[:, k] = lam_ij[i, j2]
                srl_k[:, k] = np.sign(B_ij[i, j2]) * RLSCALE
        for j2 in range(NP):
            for mh in range(MH):
                for ko in range(2):
                    i = ko * 128 + p
                    w_rows = wm[i * DS + j2, mh * 128:(mh + 1) * 128]  # [128,128]
                    w_rows = w_rows * np.abs(B_ij[i, j2])[:, None] * WSCALE
                    wdr_l[:, j2, mh, ko, :] = w_rows
        lam_a.append(lam_k)
        srl_a.append(srl_k)
        wdr_a.append(wdr_l.astype(FP8))
        bsg_a.append(np.asarray(inputs[f"b_sage{L}"], np.float32)
                     .reshape(MH, 128).T)
        ws = np.zeros((2, 2, 128, H), np.float32)
        wsf = np.asarray(inputs[f"w_self{L}"], np.float32)
        wnf = np.asarray(inputs[f"w_neigh{L}"], np.float32)
        for cc in range((f + 127) // 128):
            ws[0, cc] = wsf[cc * 128:(cc + 1) * 128]
            ws[1, cc] = wnf[cc * 128:(cc + 1) * 128]
        wsage_a.append(ws.astype(BF16))
        wr = np.zeros((2, 128, H), np.float32)
        wrf = np.asarray(inputs[f"w_res{L}"], np.float32) * (RLSCALE * WSCALE)
        for cc in range((f + 127) // 128):
            wr[cc] = wrf[cc * 128:(cc + 1) * 128]
        wres_a.append(wr.astype(BF16))
        bmr_a.append((np.asarray(inputs[f"b_res{L}"], np.float32)
                      + np.asarray(inputs[f"b_mix{L}"], np.float32))
                     .reshape(MH, 128).T)

    wout = np.asarray(inputs["w_out"], np.float32).reshape(MH, 128, CO).astype(BF16)
    bout = np.asarray(inputs["b_out"], np.float32).reshape(CO, 1)
    wcol = np.arange(64)

    in_maps = []
    for c in range(NCORES):
        onehot = (scol[c][..., None] == wcol)                 # [T, nchunk, 128, 64]
        s8 = np.ascontiguousarray(
            onehot.transpose(0, 2, 1, 3).reshape(T, 128, nchunk * 64)
        ).astype(FP8)
        sh = xs[:, c * VLOC:(c + 1) * VLOC, :]
        xs_sh = np.zeros((T, CIN, VL), np.float32)
        xs_sh[:, :, :VLOC] = np.transpose(sh, (0, 2, 1))
        ivd = np.broadcast_to(invdeg[:, c][:, None, :], (T, 128, VL))
        in_maps.append({
            "xs_in": xs_sh.astype(BF16),
            "idx_in": pack_gather_idx(cfg, src_rows[c]),
            "s8_in": s8,
            "ivd_in": np.ascontiguousarray(ivd).astype(BF16),
            "wpre_diag_in": wpre_diag,
            "bpre_in": bpre,
            "ident_in": ident,
            "lam_in": np.stack(lam_a),
            "srl_in": np.stack(srl_a),
            "bsg_in": np.stack(bsg_a).astype(np.float32),
            "wdr_in": np.stack(wdr_a),
            "wsage_in": np.stack(wsage_a),
            "wres_in": np.stack(wres_a),
            "bmr_in": np.stack(bmr_a).astype(np.float32),
            "wout_in": wout,
            "bout_in": bout,
        })
    return in_maps


_CACHED = {}


def kernel(**inputs):
    cfg = Cfg()
    in_maps = make_inputs(cfg, inputs)
    key = ("full", cfg.ncb)
    if key not in _CACHED:
        _CACHED[key] = build_program(cfg)
    nc = _CACHED[key]
    res = run_bass_kernel_spmd(nc, in_maps, list(range(NCORES)))
    out = np.zeros((cfg.V, cfg.CO), np.float32)
    for c in range(NCORES):
        out[c * cfg.VLOC:(c + 1) * cfg.VLOC] = \
            res.results[c]["out_fm"][:, :cfg.VLOC].T
    return out


# revision 10
# speedup vs baseline: 1.4460x; 1.0913x over previous
"""Trainium2 Bass kernel for nn_DiagonalSSM (token-mix -> 2x [SAGE + diagonal SSM scan] -> proj).

Sharding: nodes (V) split across 8 cores; per-core dst-sorted edge chunks with
one-hot fp8 S tiles drive the segment-mean as PE matmuls on dma-gathered
source rows; inverse-degree applied afterwards on the mean via a broadcast
tensor_tensor multiply.

SSM state [128, 16 pairs, 2*VL] bf16 ordered k=(j2, m) so the per-step h add
is ONE broadcast tensor_tensor over the whole state (h appears once per pair).
The lam multiply is 32 per-tile tensor_scalars (lam varies per (k,p)). relu
emits fp8 tiles consumed by fp8 DoubleRow matmuls (2 k-tiles of contraction
per instruction, 0.5 cyc/row) with power-of-two scaling: wmix*128, rl*8,
descale 2^-10 folded into the PSUM->SBUF copy; w_res is scaled *1024 in bf16
so residual and mix share one PSUM accumulation group.
"""

import contextlib

import numpy as np
import ml_dtypes

import concourse.bacc as bacc
import concourse.bass as bass
import concourse.mybir as mybir
import concourse.tile as tile
from concourse.bass_utils import run_bass_kernel_spmd

BF16 = ml_dtypes.bfloat16
FP8 = ml_dtypes.float8_e4m3

NCORES = 8
RLSCALE = 8.0
WSCALE = 128.0
YDESCALE = 1.0 / (RLSCALE * WSCALE)


class Cfg:
    def __init__(self, T=8, V=10000, E=100000, CIN=128, H=256, DS=16, CO=64):
        self.T, self.V, self.E = T, V, E
        self.CIN, self.H, self.DS, self.CO = CIN, H, DS, CO
        self.VLOC = V // NCORES                      # real nodes per core
        self.VL = ((self.VLOC + 127) // 128) * 128   # padded local nodes
        self.NB = self.VL // 64                      # 64-node dst blocks
        self.ncb = None                              # chunks per block (set by prep)
        self.VCS = []                                # v-chunk windows (<=512)
        off = 0
        while off < self.VL:
            w = min(512, self.VL - off)
            self.VCS.append((off, w))
            off += w
        self.K = (H * DS) // 128                     # state tiles
        self.NP = self.K // 2                        # (j2) pairs
        self.MH = H // 128                           # output chunks of H
        self.CC = {0: max(1, CIN // 128), 1: H // 128}
        # engine split knobs
        self.R_DVE = 0      # relus per 8 k-tiles on DVE
        self.R_POOL = 2     # relus per 8 k-tiles on GPSIMD (rest on Act)
        self.TS1_ACT = 3    # L1 lam-mults per 8 k-tiles on Act (rest DVE)
        self.NGRP = 4       # pair-groups per state update (pipelining)


# ----------------------------------------------------------------------------
# host-side preparation
# ----------------------------------------------------------------------------

def prep_edges(cfg, edge_index):
    T = cfg.T
    VLOC, VL, NB = cfg.VLOC, cfg.VL, cfg.NB
    ei = np.asarray(edge_index)
    src_all, dst_all = ei[:, 0, :].astype(np.int64), ei[:, 1, :].astype(np.int64)

    buckets = [[[None] * NB for _ in range(T)] for _ in range(NCORES)]
    deg = np.zeros((T, NCORES, VL), np.float32)
    for t in range(T):
        s_t, d_t = src_all[t], dst_all[t]
        core = np.minimum(d_t // VLOC, NCORES - 1)
        for c in range(NCORES):
            m = core == c
            s_c, d_c = s_t[m], d_t[m] - c * VLOC
            np.add.at(deg[t, c], d_c, 1.0)
            b_c = d_c // 64
            order = np.argsort(b_c, kind="stable")
            s_c, d_c, b_c = s_c[order], d_c[order], b_c[order]
            bounds = np.searchsorted(b_c, np.arange(NB + 1))
            for b in range(NB):
                lo, hi = bounds[b], bounds[b + 1]
                buckets[c][t][b] = (s_c[lo:hi], d_c[lo:hi])

    ncb = 1
    for c in range(NCORES):
        for t in range(T):
            for b in range(NB):
                ncb = max(ncb, (len(buckets[c][t][b][0]) + 127) // 128)
    cfg.ncb = ncb
    nchunk = NB * ncb
    invdeg = 1.0 / np.maximum(deg, 1.0)

    src_rows = np.zeros((NCORES, T, nchunk, 128), np.int16)
    scol = np.full((NCORES, T, nchunk, 128), -1, np.int64)
    for c in range(NCORES):
        for t in range(T):
            for b in range(NB):
                s_b, d_b = buckets[c][t][b]
                n = len(s_b)
                pad = ncb * 128 - n
                rows = (s_b // VLOC) * VL + (s_b % VLOC)
                rows = np.concatenate([rows, np.zeros(pad, np.int64)])
                col = np.concatenate([d_b - b * 64, np.full(pad, -1, np.int64)])
                cs = b * ncb
                src_rows[c, t, cs:cs + ncb] = rows.reshape(ncb, 128).astype(np.int16)
                scol[c, t, cs:cs + ncb] = col.reshape(ncb, 128)
    return src_rows, scol, invdeg


def pack_gather_idx(cfg, src_rows):
    # [T, 128, G*64] int16; 1024 idxs per gather call = 16 partitions x 64
    # cols, replicated 8x across the 128 partitions.
    T = cfg.T
    nchunk = src_rows.shape[1]
    G = (nchunk + 7) // 8
    out = np.zeros((T, 128, G * 64), np.int16)
    for t in range(T):
        flat = np.zeros(G * 1024, np.int16)
        flat[:nchunk * 128] = src_rows[t].reshape(-1)
        out[t] = np.tile(flat.reshape(-1, 16).T, (8, 1))
    return out


# ----------------------------------------------------------------------------
# device program
# ----------------------------------------------------------------------------

def build_program(cfg, sim1=False):
    T, VL, CIN, H, DS, CO = cfg.T, cfg.VL, cfg.CIN, cfg.H, cfg.DS, cfg.CO
    K, NP, MH = cfg.K, cfg.NP, cfg.MH
    nchunk = cfg.NB * cfg.ncb
    G = (nchunk + 7) // 8
    fp32, bf16 = mybir.dt.float32, mybir.dt.bfloat16
    fp8, i16 = mybir.dt.float8e4, mybir.dt.int16
    AT = mybir.ActivationFunctionType
    OP = mybir.AluOpType
    DR = mybir.MatmulPerfMode.DoubleRow

    ndev = 1 if sim1 else NCORES
    nc = bacc.Bacc("TRN2", target_bir_lowering=False, debug=False,
                   num_devices=ndev)

    xs_in = nc.dram_tensor("xs_in", [T, CIN, VL], bf16, kind="ExternalInput")
    idx_in = nc.dram_tensor("idx_in", [T, 128, G * 64], i16, kind="ExternalInput")
    s8_in = nc.dram_tensor("s8_in", [T, 128, nchunk * 64], fp8, kind="ExternalInput")
    ivd_in = nc.dram_tensor("ivd_in", [T, 128, VL], bf16, kind="ExternalInput")
    wpre_diag_in = nc.dram_tensor("wpre_diag_in", [3, CIN, CIN], bf16, kind="ExternalInput")
    bpre_in = nc.dram_tensor("bpre_in", [CIN, 1], fp32, kind="ExternalInput")
    ident_in = nc.dram_tensor("ident_in", [128, 128], bf16, kind="ExternalInput")
    lam_in = nc.dram_tensor("lam_in", [2, 128, K], fp32, kind="ExternalInput")
    srl_in = nc.dram_tensor("srl_in", [2, 128, K], fp32, kind="ExternalInput")
    bsg_in = nc.dram_tensor("bsg_in", [2, 128, MH], fp32, kind="ExternalInput")
    wdr_in = nc.dram_tensor("wdr_in", [2, 128, NP, MH, 2, 128], fp8, kind="ExternalInput")
    wsage_in = nc.dram_tensor("wsage_in", [2, 2, 2, 128, H], bf16, kind="ExternalInput")
    wres_in = nc.dram_tensor("wres_in", [2, 2, 128, H], bf16, kind="ExternalInput")
    bmr_in = nc.dram_tensor("bmr_in", [2, 128, MH], fp32, kind="ExternalInput")
    wout_in = nc.dram_tensor("wout_in", [2, 128, CO], bf16, kind="ExternalInput")
    bout_in = nc.dram_tensor("bout_in", [64, 1], fp32, kind="ExternalInput")

    out_fm = nc.dram_tensor("out_fm", [CO, VL], fp32, kind="ExternalOutput")

    x0_T = nc.dram_tensor("x0_T", [T, 128, VL], bf16)
    x1_T = nc.dram_tensor("x1_T", [T, 128, MH, VL], bf16)
    x0_nm = nc.dram_tensor("x0_nm", [T, VL, CIN], bf16)
    x1_nm = nc.dram_tensor("x1_nm", [T, VL, H], bf16)
    x0_full = nc.dram_tensor("x0_full", [T, NCORES * VL, CIN], bf16,
                             addr_space="Shared")
    x1_full = nc.dram_tensor("x1_full", [T, NCORES * VL, H], bf16,
                             addr_space="Shared")

    rg = [list(range(NCORES))]

    with tile.TileContext(nc) as tc, contextlib.ExitStack() as ctx:
        wpool = ctx.enter_context(tc.tile_pool(name="weights", bufs=1))
        lpool = ctx.enter_context(tc.tile_pool(name="layerw", bufs=1))
        spool = ctx.enter_context(tc.tile_pool(name="state", bufs=1))
        gpool = ctx.enter_context(tc.tile_pool(name="gather", bufs=3))
        ipool = ctx.enter_context(tc.tile_pool(name="idx", bufs=2))
        spool8 = ctx.enter_context(tc.tile_pool(name="stiles", bufs=2))
        xpool = ctx.enter_context(tc.tile_pool(name="xt", bufs=3))
        hpool = ctx.enter_context(tc.tile_pool(name="hb", bufs=3))
        rpool = ctx.enter_context(tc.tile_pool(name="rl", bufs=1))
        mpool = ctx.enter_context(tc.tile_pool(name="misc", bufs=1))
        npool = ctx.enter_context(tc.tile_pool(name="nm", bufs=1))
        pp_y = ctx.enter_context(tc.tile_pool(name="py", bufs=1, space="PSUM"))
        pp_h = ctx.enter_context(tc.tile_pool(name="ph", bufs=1, space="PSUM"))
        pp_a = ctx.enter_context(tc.tile_pool(name="pa", bufs=1, space="PSUM"))

        # ---- persistent small weights
        wpre_d = wpool.tile([CIN, 3, CIN], bf16, tag="wpred")
        for tap in range(3):
            nc.sync.dma_start(out=wpre_d[:, tap, :], in_=wpre_diag_in[tap])
        bpre = wpool.tile([CIN, 1], fp32, tag="bpre")
        nc.sync.dma_start(out=bpre[:], in_=bpre_in[:])
        ident = wpool.tile([128, 128], bf16, tag="ident")
        nc.sync.dma_start(out=ident[:], in_=ident_in[:])
        lam_t = wpool.tile([128, 2, K], fp32, tag="lamt")
        srl_t = wpool.tile([128, 2, K], fp32, tag="srlt")
        bsg_t = wpool.tile([128, 2, MH], fp32, tag="bsgt")
        for L in range(2):
            nc.sync.dma_start(out=lam_t[:, L, :], in_=lam_in[L])
            nc.sync.dma_start(out=srl_t[:, L, :], in_=srl_in[L])
            nc.sync.dma_start(out=bsg_t[:, L, :], in_=bsg_in[L])
        wsage = wpool.tile([128, 2, 2, 2, H], bf16, tag="wsage")
        for L in range(2):
            for sn in range(2):
                for cc in range(2):
                    nc.sync.dma_start(out=wsage[:, L, sn, cc, :],
                                      in_=wsage_in[L, sn, cc])
        wres = wpool.tile([128, 2, 2, H], bf16, tag="wres")
        for L in range(2):
            for cc in range(2):
                nc.sync.dma_start(out=wres[:, L, cc, :], in_=wres_in[L, cc])
        bmr = wpool.tile([128, 2, MH], fp32, tag="bmr")
        for L in range(2):
            nc.sync.dma_start(out=bmr[:, L, :], in_=bmr_in[L])
        wout = wpool.tile([128, 2, CO], bf16, tag="wout")
        for cc in range(2):
            nc.sync.dma_start(out=wout[:, cc, :], in_=wout_in[cc])
        bout = wpool.tile([64, 1], fp32, tag="bout")
        nc.sync.dma_start(out=bout[:], in_=bout_in[:])

        # ---- state; its flat bf16 view doubles as phase-A xs staging
        state = spool.tile([128, NP, 2 * VL], bf16, tag="state")
        sflat = state[:].rearrange("p a b -> p (a b)")

        # ---- phase A: token mix into x0sb, stage nm tables, AllGather per t
        for t in range(T):
            nc.sync.dma_start(out=sflat[:, t * VL:(t + 1) * VL], in_=xs_in[t])
        for t in range(T):
            sl = lambda u: sflat[:, u * VL:(u + 1) * VL]
            x0t = xpool.tile([CIN, VL], bf16, tag="x0t")
            for (woff, wlen) in cfg.VCS:
                tm_ps = pp_y.tile([128, 512], fp32, tag="ya0", name="tmps")
                taps = [(tap, t + tap - 1) for tap in range(3)
                        if 0 <= t + tap - 1 < T]
                for i, (tap, u) in enumerate(taps):
                    nc.tensor.matmul(
                        out=tm_ps[:CIN, :wlen], lhsT=wpre_d[:, tap, :],
                        rhs=sl(u)[:, woff:woff + wlen],
                        start=(i == 0), stop=(i == len(taps) - 1))
                nc.scalar.activation(x0t[:, woff:woff + wlen],
                                     tm_ps[:CIN, :wlen], AT.Identity,
                                     bias=bpre[:, 0:1], scale=1.0)
            nc.sync.dma_start(out=x0_T[t], in_=x0t[:])
            nm = npool.tile([128, VL // 128, CIN], bf16, tag="nm")
            for bi in range(VL // 128):
                pt = pp_y.tile([128, 256], bf16, tag=f"yb{bi % 2}", name="pt")
                nc.tensor.transpose(out=pt[:, :128],
                                    in_=x0t[:, bi * 128:(bi + 1) * 128],
                                    identity=ident[:])
                nc.vector.tensor_copy(nm[:, bi, :CIN], pt[:, :CIN])
            nc.sync.dma_start(out=x0_nm[t].rearrange("(b p) c -> p b c", p=128),
                              in_=nm[:])
            if sim1:
                nc.sync.dma_start(out=x0_full[t, :VL, :], in_=x0_nm[t][:])
            else:
                nc.gpsimd.collective_compute(
                    "AllGather", OP.bypass, replica_groups=rg,
                    ins=[x0_nm[t][:]], outs=[x0_full[t][:]])

        def scan_layer(L):
            CC = cfg.CC[L]
            Cin = CIN if L == 0 else H
            xfull = x0_full if L == 0 else x1_full
            wdr = lpool.tile([128, NP, MH, 2, 128], fp8, tag="wdr")
            nc.sync.dma_start(out=wdr[:], in_=wdr_in[L])

            def prepare(t):
                # xt: feature-major input for res/self matmuls
                xt = xpool.tile([128, MH, VL], bf16, tag="xin", name="xt")
                if L == 0:
                    nc.sync.dma_start(out=xt[:, 0, :], in_=x0_T[t])
                else:
                    nc.sync.dma_start(out=xt[:], in_=x1_T[t])
                # gather + one-hot aggregation; h assembled per 512-window
                idx = ipool.tile([128, G * 64], i16, tag="idx", name="idx")
                nc.sync.dma_start(out=idx[:], in_=idx_in[t])
                s8 = spool8.tile([128, nchunk * 64], fp8, tag="soh", name="s8")
                nc.sync.dma_start(out=s8[:], in_=s8_in[t])
                ivd = ipool.tile([128, VL], bf16, tag="ivd", name="ivd")
                nc.sync.dma_start(out=ivd[:], in_=ivd_in[t])
                mean_sb = xpool.tile([128, 2, VL], bf16, tag="mean", name="mean_sb")
                h_sb = hpool.tile([128, 2 * VL], bf16, tag="hsb", name="h_sb")
                win_of = {}
                for (woff, wlen) in cfg.VCS:
                    for b0 in range(woff // 64, (woff + wlen) // 64):
                        win_of[b0] = (woff, wlen)

                def finish_window(woff, wlen):
                    # mean(w) = agg(w) * invdeg(w); h(m, w) right behind
                    for cc in range(CC):
                        nc.scalar.activation(
                            mean_sb[:, cc, woff:woff + wlen],
                            agg_ps[cc][:, :wlen], AT.Copy)
                    ms = mean_sb[:, :CC, woff:woff + wlen]
                    nc.vector.tensor_tensor(
                        out=ms, in0=ms,
                        in1=ivd[:, woff:woff + wlen].unsqueeze(1)
                            .broadcast_to([128, CC, wlen]),
                        op=OP.mult)
                    for m in range(MH):
                        h_ps = pp_h.tile([128, 512], fp32, tag="h0", name="h_ps")
                        for cc in range(CC):
                            nc.tensor.matmul(
                                out=h_ps[:, :wlen],
                                lhsT=wsage[:, L, 0, cc, m * 128:(m + 1) * 128],
                                rhs=xt[:, cc if L else 0, woff:woff + wlen],
                                start=(cc == 0), stop=False)
                        for cc in range(CC):
                            nc.tensor.matmul(
                                out=h_ps[:, :wlen],
                                lhsT=wsage[:, L, 1, cc, m * 128:(m + 1) * 128],
                                rhs=mean_sb[:, cc, woff:woff + wlen],
                                start=False, stop=(cc == CC - 1))
                        nc.scalar.activation(
                            h_sb[:, m * VL + woff:m * VL + woff + wlen],
                            h_ps[:, :wlen], AT.Identity,
                            bias=bsg_t[:, L, m:m + 1], scale=1.0)

                gt = None
                agg_ps = {}
                for ch in range(nchunk):
                    g, cg = divmod(ch, 8)
                    if cg == 0:
                        gt = gpool.tile([128, 8, Cin], bf16, tag="g", name="gt")
                        nc.gpsimd.dma_gather(
                            out_ap=gt[:], in_ap=xfull[t][:],
                            idxs_ap=idx[:, g * 64:(g + 1) * 64],
                            num_idxs=1024, num_idxs_reg=1024,
                            elem_size=Cin)
                    b, cb = divmod(ch, cfg.ncb)
                    woff, wlen = win_of[b]
                    if b % 8 == 0 and cb == 0:
                        agg_ps[0] = pp_a.tile([128, 512], fp32,
                                              tag="agg0", name="agg0")
                        if CC > 1:
                            # share the ya1 PSUM banks: L1 emits y only at
                            # t=7, after its prepare's aggs are done
                            agg_ps[1] = pp_y.tile([128, 512], fp32,
                                                  tag="ya1", name="agg1")
                    boff = b * 64 - woff
                    for cc in range(CC):
                        nc.tensor.matmul(
                            out=agg_ps[cc][:, boff:boff + 64],
                            lhsT=gt[:, cg, cc * 128:(cc + 1) * 128],
                            rhs=s8[:, ch * 64:(ch + 1) * 64], start=(cb == 0),
                            stop=(cb == cfg.ncb - 1))
                    if b == (woff + wlen) // 64 - 1 and cb == cfg.ncb - 1:
                        finish_window(woff, wlen)
                return xt, h_sb

            def state_k(k):
                j2, m = divmod(k, 2)
                return state[:, j2, m * VL:(m + 1) * VL]

            pres = {0: prepare(0), 1: prepare(1)}
            h_prev = None
            for t in range(T):
                emit_y = (L == 0) or (t == T - 1)
                xt, h_sb = pres.pop(t)
                if t + 2 < T:
                    pres[t + 2] = prepare(t + 2)

                # ---- state update + relu + DR mixes, pipelined over
                # pair-groups (t=0 is implicit: state_0 = h_0)
                if emit_y:
                    yA = [pp_y.tile([128, 1024], fp32, tag=f"ya{m}", name=f"ya{m}")
                          for m in range(MH)]
                    yB = [pp_y.tile([128, 256], fp32, tag=f"yb{m}", name=f"yb{m}")
                          for m in range(MH)]
                    def ypsw(m, woff, wlen):
                        if woff + wlen <= 1024:
                            return yA[m][:, woff:woff + wlen]
                        return yB[m][:, :wlen]
                    for m in range(MH):
                        for (woff, wlen) in cfg.VCS:
                            for cc in range(CC):
                                nc.tensor.matmul(
                                    out=ypsw(m, woff, wlen),
                                    lhsT=wres[:, L, cc, m * 128:(m + 1) * 128],
                                    rhs=xt[:, cc if L else 0,
                                           woff:woff + wlen],
                                    start=(cc == 0), stop=False)
                PG = NP // cfg.NGRP
                for g in range(cfg.NGRP):
                    j2s = range(g * PG, (g + 1) * PG)
                    if t == 1:
                        for j2 in j2s:
                            for ko in range(2):
                                k = 2 * j2 + ko
                                nc.vector.tensor_scalar(
                                    state_k(k),
                                    h_prev[:, ko * VL:(ko + 1) * VL],
                                    lam_t[:, L, k:k + 1], None, OP.mult)
                    elif t > 1:
                        for j2 in j2s:
                            for ko in range(2):
                                k = 2 * j2 + ko
                                if L == 1 and (k % 8) < cfg.TS1_ACT:
                                    nc.scalar.activation(
                                        state_k(k), state_k(k), AT.Copy,
                                        scale=lam_t[:, L, k:k + 1])
                                else:
                                    nc.vector.tensor_scalar(
                                        state_k(k), state_k(k),
                                        lam_t[:, L, k:k + 1], None, OP.mult)
                    if t > 0:
                        st3 = state[:, g * PG:(g + 1) * PG, :]
                        nc.vector.tensor_tensor(
                            out=st3, in0=st3,
                            in1=h_sb[:].unsqueeze(1)
                                .broadcast_to([128, PG, 2 * VL]),
                            op=OP.add)
                    if not emit_y:
                        continue
                    for j2 in j2s:
                        rl = rpool.tile([128, 2, VL], fp8, tag=f"rl{j2 % 3}",
                                        name="rl")
                        for ko in range(2):
                            k = 2 * j2 + ko
                            src = (h_sb[:, ko * VL:(ko + 1) * VL] if t == 0
                                   else state_k(k))
                            kr = k % 8
                            if kr < cfg.R_DVE:
                                nc.vector.tensor_scalar(
                                    rl[:, ko, :], src, srl_t[:, L, k:k + 1],
                                    0.0, OP.mult, OP.max)
                            elif kr < cfg.R_DVE + cfg.R_POOL:
                                nc.gpsimd.tensor_scalar(
                                    rl[:, ko, :], src, srl_t[:, L, k:k + 1],
                                    0.0, OP.mult, OP.max)
                            else:
                                nc.scalar.activation(
                                    rl[:, ko, :], src, AT.Relu,
                                    scale=srl_t[:, L, k:k + 1])
                        for m in range(MH):
                            for (woff, wlen) in cfg.VCS:
                                nc.tensor.matmul(
                                    out=ypsw(m, woff, wlen),
                                    lhsT=wdr[:, j2, m], perf_mode=DR,
                                    rhs=rl[:, :, woff:woff + wlen],
                                    start=False, stop=(j2 == NP - 1))
                if emit_y:
                    ys = mpool.tile([128, MH, VL], bf16, tag="ys")
                    for m in range(MH):
                        nc.scalar.activation(
                            ys[:, m, :1024], yA[m][:], AT.Identity,
                            bias=bmr[:, L, m:m + 1], scale=YDESCALE)
                        nc.scalar.activation(
                            ys[:, m, 1024:], yB[m][:, :VL - 1024], AT.Identity,
                            bias=bmr[:, L, m:m + 1], scale=YDESCALE)

                if L == 0:
                    nc.sync.dma_start(out=x1_T[t], in_=ys[:])
                    nm = npool.tile([128, VL // 128, H], bf16, tag="nm")
                    for bi in range(VL // 128):
                        for m in range(MH):
                            pt = pp_y.tile([128, 256], bf16,
                                           tag=f"yb{(bi * MH + m) % 2}", name="pt")
                            nc.tensor.transpose(
                                out=pt[:, :128], in_=ys[:, m, bi * 128:(bi + 1) * 128],
                                identity=ident[:])
                            nc.vector.tensor_copy(
                                nm[:, bi, m * 128:(m + 1) * 128], pt[:, :128])
                    nc.sync.dma_start(
                        out=x1_nm[t].rearrange("(b p) c -> p b c", p=128),
                        in_=nm[:])
                    if sim1:
                        nc.sync.dma_start(out=x1_full[t, :VL, :], in_=x1_nm[t][:])
                    else:
                        nc.gpsimd.collective_compute(
                            "AllGather", OP.bypass, replica_groups=rg,
                            ins=[x1_nm[t][:]], outs=[x1_full[t][:]])
                if L == 1 and emit_y:
                    ot = mpool.tile([CO, VL], fp32, tag="outt")
                    for (woff, wlen) in cfg.VCS:
                        o_ps = pp_h.tile([CO, 512], fp32, tag="h0", name="o_ps")
                        for cc in range(MH):
                            nc.tensor.matmul(
                                out=o_ps[:, :wlen], lhsT=wout[:, cc, :],
                                rhs=ys[:, cc, woff:woff + wlen],
                                start=(cc == 0), stop=(cc == MH - 1))
                        nc.scalar.activation(ot[:, woff:woff + wlen],
                                             o_ps[:, :wlen], AT.Identity,
                                             bias=bout[:, 0:1], scale=1.0)
                    nc.sync.dma_start(out=out_fm[:], in_=ot[:])

                h_prev = h_sb
        scan_layer(0)
        scan_layer(1)

    nc.compile()
    return nc


# ----------------------------------------------------------------------------
# host wrapper
# ----------------------------------------------------------------------------

def make_inputs(cfg, inputs):
    T, CIN, H, DS, CO = cfg.T, cfg.CIN, cfg.H, cfg.DS, cfg.CO
    VLOC, VL, K, NP, MH = cfg.VLOC, cfg.VL, cfg.K, cfg.NP, cfg.MH
    xs = np.asarray(inputs["xs"], np.float32)
    src_rows, scol, invdeg = prep_edges(cfg, inputs["edge_index"])
    nchunk = cfg.NB * cfg.ncb

    w_pre = np.asarray(inputs["w_pre"], np.float32)
    wpre_diag = np.stack([np.diag(w_pre[:, tap]) for tap in range(3)]).astype(BF16)
    bpre = np.asarray(inputs["b_pre"], np.float32).reshape(CIN, 1)
    ident = np.eye(128, dtype=np.float32).astype(BF16)

    lam_a, srl_a, bsg_a, wdr_a, wsage_a, wres_a, bmr_a = [], [], [], [], [], [], []
    for L, f in ((0, CIN), (1, H)):
        lam_ij = np.exp(-np.exp(np.asarray(inputs[f"a_log{L}"], np.float64))) \
            .astype(np.float32)                                   # [H, DS]
        B_ij = np.asarray(inputs[f"B{L}"], np.float32)            # [H, DS]
        wm = np.asarray(inputs[f"w_mix{L}"], np.float32)          # [H*DS, H]
        # k = 2*j2 + m ; row p of tile k holds (i = m*128+p, j = j2)
        lam_k = np.zeros((128, K), np.float32)
        srl_k = np.zeros((128, K), np.float32)
        wdr_l = np.zeros((128, NP, MH, 2, 128), np.float32)
        p = np.arange(128)
        for j2 in range(NP):
            for m in range(2):
                i = m * 128 + p
                k = 2 * j2 + m
                lam_k[:, k] = lam_ij[i, j2]
                srl_k[:, k] = np.sign(B_ij[i, j2]) * RLSCALE
        for j2 in range(NP):
            for mh in range(MH):
                for ko in range(2):
                    i = ko * 128 + p
                    w_rows = wm[i * DS + j2, mh * 128:(mh + 1) * 128]  # [128,128]
                    w_rows = w_rows * np.abs(B_ij[i, j2])[:, None] * WSCALE
                    wdr_l[:, j2, mh, ko, :] = w_rows
        lam_a.append(lam_k)
        srl_a.append(srl_k)
        wdr_a.append(wdr_l.astype(FP8))
        bsg_a.append(np.asarray(inputs[f"b_sage{L}"], np.float32)
                     .reshape(MH, 128).T)
        ws = np.zeros((2, 2, 128, H), np.float32)
        wsf = np.asarray(inputs[f"w_self{L}"], np.float32)
        wnf = np.asarray(inputs[f"w_neigh{L}"], np.float32)
        for cc in range((f + 127) // 128):
            ws[0, cc] = wsf[cc * 128:(cc + 1) * 128]
            ws[1, cc] = wnf[cc * 128:(cc + 1) * 128]
        wsage_a.append(ws.astype(BF16))
        wr = np.zeros((2, 128, H), np.float32)
        wrf = np.asarray(inputs[f"w_res{L}"], np.float32) * (RLSCALE * WSCALE)
        for cc in range((f + 127) // 128):
            wr[cc] = wrf[cc * 128:(cc + 1) * 128]
        wres_a.append(wr.astype(BF16))
        bmr_a.append((np.asarray(inputs[f"b_res{L}"], np.float32)
                      + np.asarray(inputs[f"b_mix{L}"], np.float32))
                     .reshape(MH, 128).T)

    wout = np.asarray(inputs["w_out"], np.float32).reshape(MH, 128, CO).astype(BF16)
    bout = np.asarray(inputs["b_out"], np.float32).reshape(CO, 1)
    wcol = np.arange(64)

    in_maps = []
    for c in range(NCORES):
        onehot = (scol[c][..., None] == wcol)                 # [T, nchunk, 128, 64]
        s8 = np.ascontiguousarray(
            onehot.transpose(0, 2, 1, 3).reshape(T, 128, nchunk * 64)
        ).astype(FP8)
        sh = xs[:, c * VLOC:(c + 1) * VLOC, :]
        xs_sh = np.zeros((T, CIN, VL), np.float32)
        xs_sh[:, :, :VLOC] = np.transpose(sh, (0, 2, 1))
        ivd = np.broadcast_to(invdeg[:, c][:, None, :], (T, 128, VL))
        in_maps.append({
            "xs_in": xs_sh.astype(BF16),
            "idx_in": pack_gather_idx(cfg, src_rows[c]),
            "s8_in": s8,
            "ivd_in": np.ascontiguousarray(ivd).astype(BF16),
            "wpre_diag_in": wpre_diag,
            "bpre_in": bpre,
            "ident_in": ident,
            "lam_in": np.stack(lam_a),
            "srl_in": np.stack(srl_a),
            "bsg_in": np.stack(bsg_a).astype(np.float32),
            "wdr_in": np.stack(wdr_a),
            "wsage_in": np.stack(wsage_a),
            "wres_in": np.stack(wres_a),
            "bmr_in": np.stack(bmr_a).astype(np.float32),
            "wout_in": wout,
            "bout_in": bout,
        })
    return in_maps


_CACHED = {}


def kernel(**inputs):
    cfg = Cfg()
    in_maps = make_inputs(cfg, inputs)
    key = ("full", cfg.ncb)
    if key not in _CACHED:
        _CACHED[key] = build_program(cfg)
    nc = _CACHED[key]
    res = run_bass_kernel_spmd(nc, in_maps, list(range(NCORES)))
    out = np.zeros((cfg.V, cfg.CO), np.float32)
    for c in range(NCORES):
        out[c * cfg.VLOC:(c + 1) * cfg.VLOC] = \
            res.results[c]["out_fm"][:, :cfg.VLOC].T
    return out
